# revision 1
# baseline (speedup 1.0000x reference)
"""Trainium2 Bass kernel for a 3-layer GAT (nn_GATModel_32229434589362).

Strategy (dst-sharded, edge-major S-matrix aggregation, Ant dma_gather):
  - Nodes sharded by destination across 8 cores (6250/core); each core owns
    all edges with dst in its range (host-side bucketing of edge_index only).
  - Per layer a per-node gather table holds [per-head features | a_src] (L1/L2,
    64xf32 = 256B rows) or [h3] (L3, 128xbf16 = 256B rows), AllGather-replicated.
  - Gathers use InstDMAGatherAnt (int16 idx). The 50000-row range is covered by
    splitting each block's edges into src<32768 and src>=32768 groups, gathered
    from offset views of the same table. a_dst rows (16B payload, 256B stride)
    are gathered per edge with a small-elem variant of dma_gather.
  - p = exp(leaky(asrc+adst)) with NO max subtraction (scores are O(0.5);
    softmax is shift-invariant); normalization deferred: out = z/(s+1e-16).
  - Segment reduction per 128-node block: z,s accumulate over subtiles as PE
    matmuls with lhsT = S^T (bf16 indicator built by one is_equal) and
    rhs = [p*G | p].
"""
import sys

sys.path.insert(0, "/opt/trn_rl_repo")

import numpy as np

import concourse.bass as bass
import concourse.bacc as bacc
import concourse.tile as tile
import concourse.mybir as mybir
from concourse.masks import make_identity

f32 = mybir.dt.float32
bf16 = mybir.dt.bfloat16
i16 = mybir.dt.int16
AF = mybir.ActivationFunctionType
Alu = mybir.AluOpType

# problem constants
N, E, IN_C, H = 50000, 800000, 128, 4
NCORE, P = 8, 128
NEG = 0.2
EPS = 1e-16
R12 = 64        # T1/T2 row: [feat 32 | asrc 4 | zeros] bf16 (128B)
R3 = 128        # T3 row: [h3 128] bf16                       (256B)
RAD = 64        # adst table row: [adst 4 | junk] f32, 256B stride, 16B gathered
PAD_REL = 200.0
GCHUNK = 8      # subtiles per dma_gather call


# ---------------------------------------------------------------- host side

def _make_vu(W, att, heads, c):
    return np.stack(
        [W[:, h * c:(h + 1) * c] @ att[h] for h in range(heads)], axis=1
    ).astype(np.float32)


def _wrap16(vals):
    """int16 idx list -> [128, ceil(n/16)] wrapped layout: idx i at
    [i%16, i//16], replicated down all 8 groups of 16 partitions."""
    n = vals.shape[0]
    w = -(-n // 16)
    a = np.zeros((16, w), np.int16)
    a[(np.arange(n) % 16), (np.arange(n) // 16)] = vals.astype(np.int16)
    return np.tile(a, (8, 1))


def _preprocess(edge_index, n, ncore):
    """Bucket edges by (dst core, dst block), split each bucket by src parity.

    Slot (p, s) of a block = edge list position s*128+p. Group A (src even)
    occupies subtiles [0, SA), group B (odd) [SA, SA+SB). Gather idx is
    src>>1 (< 25000, fits int16); table rows are parity-packed in pairs.
      gxa [nblk, 128, SA*8] int16  (gather idx, wrapped layout, pad=0)
      gxb [nblk, 128, SB*8] int16
      adx [nblk, 128, ST*8] int16  (local dst idx, pad=0)
      rel [nblk, 128, ST] f32      (dst_rel, pad=PAD_REL)
    """
    npc = n // ncore
    nblk = (npc + P - 1) // P
    src = np.asarray(edge_index[0], np.int64)
    dst = np.asarray(edge_index[1], np.int64)
    core = dst // npc
    dloc = dst % npc
    blk = dloc // P
    rel = (dloc % P).astype(np.float32)
    grp = (src & 1).astype(np.int64)

    nb = ncore * nblk
    key = (core * nblk + blk) * 2 + grp
    counts = np.bincount(key, minlength=nb * 2).reshape(nb, 2)
    SA = int(-(-counts[:, 0].max() // P))
    SB = int(-(-counts[:, 1].max() // P))
    ST = SA + SB

    # slot position within the block for each edge
    order = np.argsort(key, kind="stable")
    ksort = key[order]
    starts = np.zeros(nb * 2 + 1, np.int64)
    np.cumsum(counts.reshape(-1), out=starts[1:])
    slot_in_grp = np.arange(src.shape[0]) - starts[ksort]
    bucket = ksort // 2
    grp_s = ksort % 2
    pos = np.where(grp_s == 0, slot_in_grp, SA * P + slot_in_grp)
    flat = bucket * (ST * P) + pos

    gx = np.zeros(nb * ST * P, np.int64)
    rl = np.full(nb * ST * P, PAD_REL, np.float32)
    ad = np.zeros(nb * ST * P, np.int64)
    sv = src[order]
    gx[flat] = sv >> 1
    rl[flat] = rel[order]
    ad[flat] = dloc[order]

    gx = gx.reshape(ncore, nblk, ST * P)
    rl = rl.reshape(ncore, nblk, ST, P)
    ad = ad.reshape(ncore, nblk, ST * P)

    gxa = np.zeros((ncore, nblk, 128, SA * 8), np.int16)
    gxb = np.zeros((ncore, nblk, 128, max(SB, 1) * 8), np.int16)
    adx = np.zeros((ncore, nblk, 128, ST * 8), np.int16)
    for k in range(ncore):
        for b in range(nblk):
            gxa[k, b] = _wrap16(gx[k, b, :SA * P])
            if SB:
                gxb[k, b] = _wrap16(gx[k, b, SA * P:])
            adx[k, b] = _wrap16(ad[k, b])
    # rel as [nblk, 128, ST] (partition-major slots)
    rl = np.ascontiguousarray(rl.transpose(0, 1, 3, 2))
    global _preproc_debug
    _preproc_debug = {"gx": gx, "ad": ad}  # [ncore, nblk, ST*P] slot-major
    return gxa, gxb, adx, rl, SA, SB, npc, nblk


_preproc_debug = None


# ---------------------------------------------------------------- device side

def _gather_calls(gp, out3, table_even, table_odd, idxa, idxb,
                  SA, SB, elem, queue=0):
    """Exactly 2 calls per parity group (ceil/floor split, <=1024 descs each),
    queue = call-position parity so each of the 8 DMASW lanes is pinned to
    one SWDGE queue (8 SWDGE calls per block total, incl. 4 AD calls).
    out3: [128, ST, elem]; table views have elem_step = 2*elem."""
    pos = 0
    for base, cnt, tbl, idx in ((0, SA, table_even, idxa),
                                (SA, SB, table_odd, idxb)):
        c1 = (cnt + 1) // 2
        for c0, cl in ((0, c1), (c1, cnt - c1)):
            if cl <= 0:
                pos += 1
                continue
            assert cl * P <= 1024, "SWDGE ring limit"
            num = cl * P
            _dma_gather_raw(
                gp,
                out3[:, base + c0:base + c0 + cl, :],
                tbl,
                idx[:, c0 * 8:(c0 + cl) * 8],
                num, elem, 2 * elem,
                queue_num=pos % 2,
            )
            pos += 1


def _dma_gather_raw(gp, out_ap, in_ap, idxs_ap, num_idxs, elem_size,
                    elem_step, queue_num=0):
    """dma_gather (DRAM, non-transpose) minus the elem_size%256 assert — the
    Q7 ucode is size-agnostic here; only the stride must be 256B-aligned."""
    from concourse import ap_utils
    assert idxs_ap.dtype == mybir.dt.int16
    assert in_ap.dtype == out_ap.dtype
    assert ap_utils.ap_is_contiguous(out_ap.ap[1:])
    assert ap_utils.ap_is_contiguous(idxs_ap.ap[1:])
    assert in_ap.ap[0][0] == elem_step and in_ap.ap[-1][1] == elem_size
    assert out_ap.ap[-1][1] == elem_size
    assert out_ap.ap[0][1] * out_ap.ap[1][1] == num_idxs
    stride_bytes_256 = (elem_step * mybir.dt.size(in_ap.dtype)) // 256
    _in_ap = gp.lower_ap_dma(in_ap, for_custom_bir_dma=True)
    inst = gp.add_instruction(
        mybir.InstDMAGatherAnt(
            name=gp.bass.get_next_instruction_name(),
            ins=[*_in_ap, gp.lower_ap(idxs_ap),
                 gp.lower_val_access(gp.to_reg(num_idxs))],
            outs=[gp.lower_ap(out_ap)],
            transpose=False,
            num_idxs=num_idxs,
            elem_size=elem_size,
            stride_bytes_256=stride_bytes_256,
            gen_mode=0,
            single_packet=True,
            queue_num=queue_num,
            sbuf_tokens_per_rank=0,
            sbuf_free_dim_per_rank=0,
            sbuf_free_dim_pad_per_rank=0,
            sbuf_byte_offset=0,
        )
    )
    return inst


def _build_program(n, SA, SB, npc, nblk, use_collectives=True,
                   debug_tables=False):
    nc = bacc.Bacc("TRN2", num_devices=NCORE, num_swdge_queues=2)
    ST = SA + SB
    npcp = nblk * P
    WA, WB, WT = SA * 8, max(SB, 1) * 8, ST * 8

    BW = WA + WB + WT + ST   # int16 cols per block in the mega index tile
    xT = nc.dram_tensor("xT", [P, npcp], f32, kind="ExternalInput")
    mega = nc.dram_tensor("mega", [P, nblk * BW], i16, kind="ExternalInput")
    w1p = nc.dram_tensor("w1p", [P, 68], f32, kind="ExternalInput")
    w2t = nc.dram_tensor("w2t", [32, 128], f32, kind="ExternalInput")
    vu2 = nc.dram_tensor("vu2", [32, 8], f32, kind="ExternalInput")
    w3p = nc.dram_tensor("w3p", [P, 132], f32, kind="ExternalInput")
    as3r = nc.dram_tensor("as3r", [P, 128], bf16, kind="ExternalInput")
    b1r = nc.dram_tensor("b1r", [P, 32], f32, kind="ExternalInput")
    b2r = nc.dram_tensor("b2r", [P, 128], f32, kind="ExternalInput")
    b3r = nc.dram_tensor("b3r", [P, 32], f32, kind="ExternalInput")
    iot = nc.dram_tensor("iot", [P, P], bf16, kind="ExternalInput")
    out = nc.dram_tensor("out", [npc, 32], f32, kind="ExternalOutput")
    rg = [list(range(NCORE))]

    with tile.TileContext(nc) as tc:
        with tc.tile_pool(name="dramp", bufs=1, space="DRAM") as dramp, \
                tc.tile_pool(name="constp", bufs=1) as constp:
            t1loc = dramp.tile([npc, R12], bf16)
            t1full = dramp.tile([n, R12], bf16, addr_space="Shared")
            t2loc = dramp.tile([npc, R12], bf16)
            t2full = dramp.tile([n, R12], bf16, addr_space="Shared")
            t3loc = dramp.tile([npc, R3], bf16)
            t3full = dramp.tile([n, R3], bf16, addr_space="Shared")
            ad1 = dramp.tile([npc, RAD], f32)
            ad2 = dramp.tile([npc, RAD], f32)
            ad3 = dramp.tile([npc, RAD], f32)

            def cload(name, shape, dt, src):
                t = constp.tile(shape, dt, name=name)
                nc.sync.dma_start(t, src)
                return t

            w1p_s = cload("w1p_s", [P, 68], f32, w1p[:, :])
            w2t_s = cload("w2t_s", [32, 128], f32, w2t[:, :])
            vu2_s = cload("vu2_s", [32, 8], f32, vu2[:, :])
            w3p_s = cload("w3p_s", [P, 132], f32, w3p[:, :])
            as3_s = cload("as3_s", [P, 128], bf16, as3r[:, :])
            b1r_s = cload("b1r_s", [P, 32], f32, b1r[:, :])
            b2r_s = cload("b2r_s", [P, 128], f32, b2r[:, :])
            b3r_s = cload("b3r_s", [P, 32], f32, b3r[:, :])
            iot_s = cload("iot_s", [P, P], bf16, iot[:, :])
            ident = constp.tile([P, P], f32)
            make_identity(nc, ident)
            mega_s = constp.tile([P, nblk * BW], i16)
            nc.sync.dma_start(mega_s, mega[:, :])

            # parity-packed views: row pair (2g, 2g+1) -> packed row g; the
            # even/odd views address each half at stride 2*elem
            def parity_views(t, r):
                v = t.rearrange("(a b) c -> a (b c)", b=2)
                return v[:, 0:r], v[:, r:2 * r]

            t1a, t1b = parity_views(t1full, R12)
            t2a, t2b = parity_views(t2full, R12)
            t3a, t3b = parity_views(t3full, R3)

            # ---- stage A: T1 rows = [x@W1 | x@V1 | 0...], ad1 = x@U1 ----
            with tc.tile_pool(name="sa", bufs=3) as sa, \
                    tc.tile_pool(name="pa", bufs=2, space="PSUM") as pa:
                for b in range(nblk):
                    xb = sa.tile([P, P], f32, tag="xb")
                    nc.sync.dma_start(xb, xT[:, b * P:(b + 1) * P])
                    hp = pa.tile([P, 68], f32, tag="hp")
                    nc.tensor.matmul(hp, xb, w1p_s, start=True, stop=True)
                    hs = sa.tile([P, 64], bf16, tag="hs")
                    nc.vector.memset(hs[:, 36:64], 0.0)
                    nc.scalar.copy(hs[:, 0:36], hp[:, 0:36])
                    ha4 = sa.tile([P, 4], f32, tag="ha4")
                    nc.scalar.copy(ha4, hp[:, 64:68])
                    rows = min(P, npc - b * P)
                    nc.sync.dma_start(
                        t1loc[b * P:b * P + rows, :], hs[0:rows, :])
                    nc.sync.dma_start(
                        ad1[b * P:b * P + rows, 0:4], ha4[0:rows, :])

            def allgather(loc, full):
                if use_collectives:
                    nc.gpsimd.collective_compute(
                        "AllGather", Alu.bypass, replica_groups=rg,
                        ins=[loc[:, :].opt()], outs=[full[:, :].opt()])
                else:
                    nc.sync.dma_start(full[0:npc, :], loc[:, :])

            allgather(t1loc, t1full)

            if debug_tables:
                dbg1 = nc.dram_tensor("dbg1", [n, R12], bf16, kind="ExternalOutput")
                dbgad = nc.dram_tensor("dbgad", [npc, 4], f32, kind="ExternalOutput")
                dbg2 = nc.dram_tensor("dbg2", [npc, R12], bf16, kind="ExternalOutput")
                dbg3 = nc.dram_tensor("dbg3", [npc, R3], bf16, kind="ExternalOutput")
                dbgG = nc.dram_tensor("dbgG", [P, ST * R12], bf16, kind="ExternalOutput")
                dbgAD = nc.dram_tensor("dbgAD", [P, ST * 4], f32, kind="ExternalOutput")
                dbgPT = nc.dram_tensor("dbgPT", [P, ST * 4], bf16, kind="ExternalOutput")
                dbgST = nc.dram_tensor("dbgST", [P, ST * P], bf16, kind="ExternalOutput")
                dbgZN = nc.dram_tensor("dbgZN", [P, 32], f32, kind="ExternalOutput")

            def elu(sp, x_ap, cols, tag):
                mn = sp.tile([P, cols], f32, tag=tag + "mn")
                nc.vector.tensor_scalar_min(mn, x_ap, 0.0)
                ex = sp.tile([P, cols], f32, tag=tag + "ex")
                nc.scalar.activation(ex, mn, AF.Exp)
                mx = sp.tile([P, cols], f32, tag=tag + "mx")
                nc.vector.tensor_scalar_max(mx, x_ap, 0.0)
                sm = sp.tile([P, cols], f32, tag=tag + "sm")
                nc.vector.tensor_tensor(out=sm, in0=mx, in1=ex, op=Alu.add)
                res = sp.tile([P, cols], f32, tag=tag + "rs")
                nc.vector.tensor_scalar_add(res, sm, -1.0)
                return res

            def gat_layer(layer, tha, thb, adt, rpg, epilogue):
                """layer: 1/2/3. rpg: PG width (32 L1, 128 L2/L3)."""
                l3 = layer == 3
                Rt = R3 if l3 else R12
                gdt = bf16
                with tc.tile_pool(name=f"sp{layer}", bufs=4) as sp, \
                        tc.tile_pool(name=f"pp{layer}", bufs=2, space="PSUM") as pp:
                    for b in range(nblk):
                        base = b * BW
                        ixa = mega_s[:, base:base + WA]
                        ixb = mega_s[:, base + WA:base + WA + WB]
                        ixd = mega_s[:, base + WA + WB:base + WA + WB + WT]
                        relf = mega_s[:, base + WA + WB + WT:
                                      base + BW].bitcast(bf16)

                        G = sp.tile([P, ST * Rt], gdt, tag="G")
                        g3 = G.rearrange("p (s r) -> p s r", r=Rt)
                        _gather_calls(nc.gpsimd, g3, tha, thb,
                                      ixa, ixb, SA, SB, Rt, queue=0)
                        AD = sp.tile([P, ST * 4], f32, tag="AD")
                        adc = (ST + 3) // 4
                        c0 = 0
                        for j in range(4):
                            cl = min(adc, ST - c0)
                            if cl <= 0:
                                continue
                            _dma_gather_raw(
                                nc.gpsimd,
                                AD.rearrange("p (s h) -> p s h", h=4)[
                                    :, c0:c0 + cl, :],
                                adt, ixd[:, c0 * 8:(c0 + cl) * 8],
                                cl * P, 4, RAD, queue_num=j % 2)
                            c0 += cl

                        # e = leaky(asrc + adst); p = exp(e)
                        ee = sp.tile([P, ST * 4], f32, tag="ee")
                        if l3:
                            # asrc3 per edge = per-head dot(h3_row, as3)
                            gm = sp.tile([P, ST * 128], bf16, tag="gm")
                            nc.vector.tensor_tensor(
                                out=gm.rearrange("p (s h c) -> p s h c", h=4, c=32),
                                in0=g3.rearrange("p s (h c) -> p s h c", c=32),
                                in1=as3_s.rearrange(
                                    "p (h c) -> p h c", c=32).unsqueeze(1)
                                .broadcast_to([P, ST, 4, 32]),
                                op=Alu.mult)
                            ar = sp.tile([P, ST * 4], f32, tag="ar")
                            nc.vector.tensor_reduce(
                                out=ar.rearrange("p (s h) -> p s h", h=4),
                                in_=gm.rearrange("p (s h c) -> p s h c", h=4, c=32),
                                axis=mybir.AxisListType.X,
                                op=Alu.add)
                            nc.vector.tensor_tensor(
                                out=ee, in0=ar, in1=AD, op=Alu.add)
                        else:
                            nc.vector.tensor_tensor(
                                out=ee.rearrange("p (s h) -> p s h", h=4),
                                in0=g3[:, :, 32:36],
                                in1=AD.rearrange("p (s h) -> p s h", h=4),
                                op=Alu.add)
                        es = sp.tile([P, ST * 4], f32, tag="es")
                        nc.vector.tensor_scalar_mul(es, ee, NEG)
                        el = sp.tile([P, ST * 4], f32, tag="el")
                        nc.vector.tensor_tensor(out=el, in0=ee, in1=es, op=Alu.max)
                        pt = sp.tile([P, ST * 4], bf16, tag="pt")
                        nc.scalar.activation(pt, el, AF.Exp)

                        # S^T[p, s, j] = (dst_rel == j), bf16
                        st = sp.tile([P, ST * P], bf16, tag="st")
                        nc.vector.tensor_tensor(
                            out=st.rearrange("p (s j) -> p s j", j=P),
                            in0=relf.unsqueeze(2).broadcast_to([P, ST, P]),
                            in1=iot_s.unsqueeze(1).broadcast_to([P, ST, P]),
                            op=Alu.is_equal)

                        # rhs = [p*G-slices | p] bf16 per subtile
                        rw = rpg + 4
                        pgp = sp.tile([P, ST * rw], bf16, tag="pgp")
                        pg4 = pgp.rearrange("p (s m) -> p s m", m=rw)
                        pt3 = pt.rearrange("p (s h) -> p s h", h=4)
                        cw = rpg // 4
                        if layer == 1:
                            in0 = g3[:, :, 0:32].rearrange(
                                "p s (h c) -> p s h c", c=8)
                        elif layer == 2:
                            in0 = g3[:, :, 0:32].unsqueeze(2).broadcast_to(
                                [P, ST, 4, 32])
                        else:
                            in0 = g3.rearrange("p s (h c) -> p s h c", c=32)
                        nc.vector.tensor_tensor(
                            out=pg4[:, :, 0:rpg].rearrange(
                                "p s (h c) -> p s h c", c=cw),
                            in0=in0,
                            in1=pt3.unsqueeze(3).broadcast_to([P, ST, 4, cw]),
                            op=Alu.mult)
                        nc.vector.tensor_copy(out=pg4[:, :, rpg:rw], in_=pt3)

                        zb = pp.tile([P, rw], f32, tag="zb")
                        st3 = st.rearrange("p (s j) -> p s j", j=P)
                        for s in range(ST):
                            nc.tensor.matmul(
                                zb, st3[:, s, :], pg4[:, s, :],
                                start=(s == 0), stop=(s == ST - 1))

                        # znorm = z / (s + eps)
                        rr = sp.tile([P, 4], f32, tag="rr")
                        nc.vector.tensor_scalar_add(rr, zb[:, rpg:rw], EPS)
                        rr2 = sp.tile([P, 4], f32, tag="rr2")
                        nc.vector.reciprocal(rr2, rr)
                        zn = sp.tile([P, rpg], f32, tag="zn")
                        nc.vector.tensor_tensor(
                            out=zn.rearrange("p (h c) -> p h c", c=cw),
                            in0=zb[:, 0:rpg].rearrange("p (h c) -> p h c", c=cw),
                            in1=rr2.unsqueeze(2).broadcast_to([P, 4, cw]),
                            op=Alu.mult)

                        if debug_tables and layer == 1 and b == 0:
                            nc.sync.dma_start(dbgG[:, :], G)
                            nc.sync.dma_start(dbgAD[:, :], AD)
                            nc.sync.dma_start(dbgPT[:, :], pt)
                            nc.sync.dma_start(dbgST[:, :], st)
                            nc.sync.dma_start(dbgZN[:, :], zn)

                        epilogue(b, sp, pp, zn)

            # ---- layer epilogues ----
            def epi1(b, sp, pp, zn):
                tb = sp.tile([P, 32], f32, tag="tb")
                nc.vector.tensor_tensor(out=tb, in0=zn, in1=b1r_s, op=Alu.add)
                t2 = elu(sp, tb, 32, "e1")
                t2T_p = pp.tile([32, P], f32, tag="t2T_p", bufs=1)
                nc.tensor.transpose(t2T_p, t2, ident)
                t2T = sp.tile([32, P], f32, tag="t2T")
                nc.scalar.copy(t2T, t2T_p)
                av_p = pp.tile([P, 8], f32, tag="av_p", bufs=1)
                nc.tensor.matmul(av_p, t2T, vu2_s, start=True, stop=True)
                stg = sp.tile([P, 64], bf16, tag="stg")
                nc.vector.memset(stg[:, 36:64], 0.0)
                nc.scalar.copy(stg[:, 0:32], t2)
                av = sp.tile([P, 8], f32, tag="av")
                nc.scalar.copy(av, av_p)
                nc.vector.tensor_copy(out=stg[:, 32:36], in_=av[:, 0:4])
                rows = min(P, npc - b * P)
                nc.sync.dma_start(t2loc[b * P:b * P + rows, :], stg[0:rows, :])
                nc.sync.dma_start(ad2[b * P:b * P + rows, 0:4], av[0:rows, 4:8])

            def epi2(b, sp, pp, zn):
                o2 = pp.tile([P, 128], f32, tag="o2", bufs=1)
                for h in range(4):
                    zT_p = pp.tile([32, P], f32, tag="zT_p", bufs=2)
                    nc.tensor.transpose(zT_p, zn[:, 32 * h:32 * h + 32], ident)
                    zT = sp.tile([32, P], f32, tag="zT")
                    nc.scalar.copy(zT, zT_p)
                    nc.tensor.matmul(
                        o2[:, 32 * h:32 * h + 32], zT,
                        w2t_s[:, 32 * h:32 * h + 32], start=True, stop=True)
                ob = sp.tile([P, 128], f32, tag="ob")
                nc.vector.tensor_tensor(out=ob, in0=o2, in1=b2r_s, op=Alu.add)
                t3 = elu(sp, ob, 128, "e2")
                t3T_p = pp.tile([P, P], f32, tag="t3T_p", bufs=1)
                nc.tensor.transpose(t3T_p, t3, ident)
                t3T = sp.tile([P, P], f32, tag="t3T")
                nc.scalar.copy(t3T, t3T_p)
                hv_p = pp.tile([P, 132], f32, tag="hv_p", bufs=1)
                nc.tensor.matmul(hv_p, t3T, w3p_s, start=True, stop=True)
                hvb = sp.tile([P, 128], bf16, tag="hvb")
                nc.scalar.copy(hvb, hv_p[:, 0:128])
                hva = sp.tile([P, 4], f32, tag="hva")
                nc.scalar.copy(hva, hv_p[:, 128:132])
                rows = min(P, npc - b * P)
                nc.sync.dma_start(t3loc[b * P:b * P + rows, :], hvb[0:rows, :])
                nc.sync.dma_start(ad3[b * P:b * P + rows, 0:4], hva[0:rows, :])

            def epi3(b, sp, pp, zn):
                a1 = sp.tile([P, 32], f32, tag="a1")
                nc.vector.tensor_tensor(
                    out=a1, in0=zn[:, 0:32], in1=zn[:, 32:64], op=Alu.add)
                a2 = sp.tile([P, 32], f32, tag="a2")
                nc.vector.tensor_tensor(
                    out=a2, in0=a1, in1=zn[:, 64:96], op=Alu.add)
                a3 = sp.tile([P, 32], f32, tag="a3")
                nc.vector.tensor_tensor(
                    out=a3, in0=a2, in1=zn[:, 96:128], op=Alu.add)
                a4 = sp.tile([P, 32], f32, tag="a4")
                nc.vector.tensor_scalar_mul(a4, a3, 0.25)
                a5 = sp.tile([P, 32], f32, tag="a5")
                nc.vector.tensor_tensor(out=a5, in0=a4, in1=b3r_s, op=Alu.add)
                rows = min(P, npc - b * P)
                nc.sync.dma_start(out[b * P:b * P + rows, :], a5[0:rows, :])

            gat_layer(1, t1a, t1b, ad1[:, 0:4], 32, epi1)
            allgather(t2loc, t2full)
            gat_layer(2, t2a, t2b, ad2[:, 0:4], 128, epi2)
            allgather(t3loc, t3full)
            gat_layer(3, t3a, t3b, ad3[:, 0:4], 128, epi3)

            if debug_tables:
                nc.sync.dma_start(dbg1[:, :], t1full[:, :])
                nc.sync.dma_start(dbgad[:, :], ad1[:, 0:4])
                nc.sync.dma_start(dbg2[:, :], t2loc[:, :])
                nc.sync.dma_start(dbg3[:, :], t3loc[:, :])

    nc.compile()
    return nc


# ---------------------------------------------------------------- entry point

def _host_inputs(inputs, n):
    x = np.asarray(inputs["x"], np.float32)
    edge_index = np.asarray(inputs["edge_index"])
    W1 = np.asarray(inputs["W1"], np.float32)
    W2 = np.asarray(inputs["W2"], np.float32)
    W3 = np.asarray(inputs["W3"], np.float32)
    as1, ad1 = np.asarray(inputs["as1"]), np.asarray(inputs["ad1"])
    as2, ad2 = np.asarray(inputs["as2"]), np.asarray(inputs["ad2"])
    as3, ad3 = np.asarray(inputs["as3"]), np.asarray(inputs["ad3"])
    b1, b2, b3 = [np.asarray(inputs[k], np.float32) for k in ("b1", "b2", "b3")]

    gxa, gxb, adx, rl, SA, SB, npc, nblk = _preprocess(edge_index, n, NCORE)
    npcp = nblk * P

    # w1p: [W1(32) | V1(4) | zeros(28) | U1(4)] -> psum cols 0:64 = table row
    w1pack = np.concatenate(
        [W1, _make_vu(W1, as1, H, 8), np.zeros((IN_C, 28), np.float32),
         _make_vu(W1, ad1, H, 8)], axis=1)
    vu2p = np.concatenate(
        [_make_vu(W2, as2, H, 32), _make_vu(W2, ad2, H, 32)], axis=1)
    w3pack = np.concatenate([W3, _make_vu(W3, ad3, H, 32)], axis=1)
    as3rep = np.tile(np.asarray(as3, np.float32).reshape(1, 128), (P, 1))

    import ml_dtypes
    ST = SA + SB
    WA, WB, WT = SA * 8, max(SB, 1) * 8, ST * 8
    rl16 = rl.astype(ml_dtypes.bfloat16).view(np.int16)
    megas = np.concatenate([gxa, gxb, adx, rl16], axis=3)  # [nc, nblk, 128, BW]
    megas = megas.transpose(0, 2, 1, 3).reshape(NCORE, P, -1)
    iota = np.tile(np.arange(P, dtype=np.float32), (P, 1)).astype(ml_dtypes.bfloat16)
    reps = {
        "w1p": w1pack.astype(np.float32),
        "w2t": np.ascontiguousarray(W2),
        "vu2": np.ascontiguousarray(vu2p),
        "w3p": w3pack.astype(np.float32),
        "as3r": as3rep,  # cast to bf16 at dma? inputs must match dtype: cast below
        "b1r": np.tile(b1, (P, 1)).astype(np.float32),
        "b2r": np.tile(b2, (P, 1)).astype(np.float32),
        "b3r": np.tile(b3, (P, 1)).astype(np.float32),
        "iot": iota,
    }
    reps["as3r"] = as3rep.astype(ml_dtypes.bfloat16)

    in_maps = []
    for k in range(NCORE):
        xk = x[k * npc:(k + 1) * npc]
        xT = np.zeros((P, npcp), np.float32)
        xT[:, :npc] = xk.T
        m = {"xT": xT, "mega": np.ascontiguousarray(megas[k])}
        m.update(reps)
        in_maps.append(m)
    return in_maps, SA, SB, npc, nblk


def _run(inputs, n, e, trace=False):
    from concourse.bass_utils import run_bass_kernel_spmd

    in_maps, SA, SB, npc, nblk = _host_inputs(inputs, n)
    nc = _build_program(n, SA, SB, npc, nblk)
    res = run_bass_kernel_spmd(
        nc, in_maps, core_ids=list(range(NCORE)), trace=trace)
    outs = [res.results[k]["out"] for k in range(NCORE)]
    full = np.concatenate(outs, axis=0).astype(np.float32)
    return full, res


def kernel(**inputs):
    full, _ = _run(inputs, N, E, trace=False)
    return full



# revision 4
# speedup vs baseline: 1.1266x; 1.1266x over previous
"""Trainium2 Bass kernel for a 3-layer GAT (nn_GATModel_32229434589362).

Strategy (dst-sharded, edge-major S-matrix aggregation, Ant dma_gather):
  - Nodes sharded by destination across 8 cores (6250/core); each core owns
    all edges with dst in its range (host-side bucketing of edge_index only).
  - Per layer a per-node gather table holds [per-head features | a_src] (L1/L2,
    64xf32 = 256B rows) or [h3] (L3, 128xbf16 = 256B rows), AllGather-replicated.
  - Gathers use InstDMAGatherAnt (int16 idx). The 50000-row range is covered by
    splitting each block's edges into src<32768 and src>=32768 groups, gathered
    from offset views of the same table. a_dst rows (16B payload, 256B stride)
    are gathered per edge with a small-elem variant of dma_gather.
  - p = exp(leaky(asrc+adst)) with NO max subtraction (scores are O(0.5);
    softmax is shift-invariant); normalization deferred: out = z/(s+1e-16).
  - Segment reduction per 128-node block: z,s accumulate over subtiles as PE
    matmuls with lhsT = S^T (bf16 indicator built by one is_equal) and
    rhs = [p*G | p].
"""
import sys

sys.path.insert(0, "/opt/trn_rl_repo")

import numpy as np

import concourse.bass as bass
import concourse.bacc as bacc
import concourse.tile as tile
import concourse.mybir as mybir
from concourse.masks import make_identity

f32 = mybir.dt.float32
bf16 = mybir.dt.bfloat16
i16 = mybir.dt.int16
AF = mybir.ActivationFunctionType
Alu = mybir.AluOpType

# problem constants
N, E, IN_C, H = 50000, 800000, 128, 4
NCORE, P = 8, 128
NEG = 0.2
EPS = 1e-16
R12 = 64        # T1/T2 row: [feat 32 | asrc 4 | zeros] bf16 (128B)
R3 = 128        # T3 row: [h3 128] bf16                       (256B)
RAD = 64        # adst table row: [adst 4 | junk] f32, 256B stride, 16B gathered
PAD_REL = 200.0
GCHUNK = 8      # subtiles per dma_gather call


# ---------------------------------------------------------------- host side

def _make_vu(W, att, heads, c):
    return np.stack(
        [W[:, h * c:(h + 1) * c] @ att[h] for h in range(heads)], axis=1
    ).astype(np.float32)


def _wrap16(vals):
    """int16 idx list -> [128, ceil(n/16)] wrapped layout: idx i at
    [i%16, i//16], replicated down all 8 groups of 16 partitions."""
    n = vals.shape[0]
    w = -(-n // 16)
    a = np.zeros((16, w), np.int16)
    a[(np.arange(n) % 16), (np.arange(n) // 16)] = vals.astype(np.int16)
    return np.tile(a, (8, 1))


def _preprocess(edge_index, n, ncore):
    """Bucket edges by (dst core, dst block), split each bucket by src parity.

    Slot (p, s) of a block = edge list position s*128+p. Group A (src even)
    occupies subtiles [0, SA), group B (odd) [SA, SA+SB). Gather idx is
    src>>1 (< 25000, fits int16); table rows are parity-packed in pairs.
      gxa [nblk, 128, SA*8] int16  (gather idx, wrapped layout, pad=0)
      gxb [nblk, 128, SB*8] int16
      adx [nblk, 128, ST*8] int16  (local dst idx, pad=0)
      rel [nblk, 128, ST] f32      (dst_rel, pad=PAD_REL)
    """
    npc = n // ncore
    nblk = (npc + P - 1) // P
    src = np.asarray(edge_index[0], np.int64)
    dst = np.asarray(edge_index[1], np.int64)
    core = dst // npc
    dloc = dst % npc
    blk = dloc // P
    rel = (dloc % P).astype(np.float32)
    grp = (src & 1).astype(np.int64)

    nb = ncore * nblk
    key = (core * nblk + blk) * 2 + grp
    counts = np.bincount(key, minlength=nb * 2).reshape(nb, 2)
    SA = int(-(-counts[:, 0].max() // P))
    SB = int(-(-counts[:, 1].max() // P))
    ST = SA + SB

    # slot position within the block for each edge
    order = np.argsort(key, kind="stable")
    ksort = key[order]
    starts = np.zeros(nb * 2 + 1, np.int64)
    np.cumsum(counts.reshape(-1), out=starts[1:])
    slot_in_grp = np.arange(src.shape[0]) - starts[ksort]
    bucket = ksort // 2
    grp_s = ksort % 2
    pos = np.where(grp_s == 0, slot_in_grp, SA * P + slot_in_grp)
    flat = bucket * (ST * P) + pos

    gx = np.zeros(nb * ST * P, np.int64)
    rl = np.full(nb * ST * P, PAD_REL, np.float32)
    ad = np.zeros(nb * ST * P, np.int64)
    sv = src[order]
    gx[flat] = sv >> 1
    rl[flat] = rel[order]
    ad[flat] = dloc[order]

    gx = gx.reshape(ncore, nblk, ST * P)
    rl = rl.reshape(ncore, nblk, ST, P)
    ad = ad.reshape(ncore, nblk, ST * P)

    gxa = np.zeros((ncore, nblk, 128, SA * 8), np.int16)
    gxb = np.zeros((ncore, nblk, 128, max(SB, 1) * 8), np.int16)
    adx = np.zeros((ncore, nblk, 128, ST * 8), np.int16)
    for k in range(ncore):
        for b in range(nblk):
            gxa[k, b] = _wrap16(gx[k, b, :SA * P])
            if SB:
                gxb[k, b] = _wrap16(gx[k, b, SA * P:])
            adx[k, b] = _wrap16(ad[k, b])
    # rel as [nblk, 128, ST] (partition-major slots)
    rl = np.ascontiguousarray(rl.transpose(0, 1, 3, 2))
    global _preproc_debug
    _preproc_debug = {"gx": gx, "ad": ad}  # [ncore, nblk, ST*P] slot-major
    return gxa, gxb, adx, rl, SA, SB, npc, nblk


_preproc_debug = None


# ---------------------------------------------------------------- device side

def _gather_calls(gp, out3, table_even, table_odd, idxa, idxb,
                  SA, SB, elem, queue=0):
    """Exactly 2 calls per parity group (ceil/floor split, <=1024 descs each),
    queue = call-position parity so each of the 8 DMASW lanes is pinned to
    one SWDGE queue (8 SWDGE calls per block total, incl. 4 AD calls).
    out3: [128, ST, elem]; table views have elem_step = 2*elem."""
    pos = 0
    for base, cnt, tbl, idx in ((0, SA, table_even, idxa),
                                (SA, SB, table_odd, idxb)):
        c1 = (cnt + 1) // 2
        for c0, cl in ((0, c1), (c1, cnt - c1)):
            if cl <= 0:
                pos += 1
                continue
            assert cl * P <= 1024, "SWDGE ring limit"
            num = cl * P
            _dma_gather_raw(
                gp,
                out3[:, base + c0:base + c0 + cl, :],
                tbl,
                idx[:, c0 * 8:(c0 + cl) * 8],
                num, elem, 2 * elem,
                queue_num=1 + pos % 3,
            )
            pos += 1


def _dma_gather_raw(gp, out_ap, in_ap, idxs_ap, num_idxs, elem_size,
                    elem_step, queue_num=0):
    """dma_gather (DRAM, non-transpose) minus the elem_size%256 assert — the
    Q7 ucode is size-agnostic here; only the stride must be 256B-aligned."""
    from concourse import ap_utils
    assert idxs_ap.dtype == mybir.dt.int16
    assert in_ap.dtype == out_ap.dtype
    assert ap_utils.ap_is_contiguous(out_ap.ap[1:])
    assert ap_utils.ap_is_contiguous(idxs_ap.ap[1:])
    assert in_ap.ap[0][0] == elem_step and in_ap.ap[-1][1] == elem_size
    assert out_ap.ap[-1][1] == elem_size
    assert out_ap.ap[0][1] * out_ap.ap[1][1] == num_idxs
    stride_bytes_256 = (elem_step * mybir.dt.size(in_ap.dtype)) // 256
    _in_ap = gp.lower_ap_dma(in_ap, for_custom_bir_dma=True)
    inst = gp.add_instruction(
        mybir.InstDMAGatherAnt(
            name=gp.bass.get_next_instruction_name(),
            ins=[*_in_ap, gp.lower_ap(idxs_ap),
                 gp.lower_val_access(gp.to_reg(num_idxs))],
            outs=[gp.lower_ap(out_ap)],
            transpose=False,
            num_idxs=num_idxs,
            elem_size=elem_size,
            stride_bytes_256=stride_bytes_256,
            gen_mode=0,
            single_packet=True,
            queue_num=queue_num,
            sbuf_tokens_per_rank=0,
            sbuf_free_dim_per_rank=0,
            sbuf_free_dim_pad_per_rank=0,
            sbuf_byte_offset=0,
        )
    )
    return inst


def _build_program(n, SA, SB, npc, nblk, use_collectives=True,
                   debug_tables=False):
    nc = bacc.Bacc("TRN2", num_devices=NCORE, num_swdge_queues=4)
    ST = SA + SB
    npcp = nblk * P
    WA, WB, WT = SA * 8, max(SB, 1) * 8, ST * 8

    BW = WA + WB + WT + ST   # int16 cols per block in the mega index tile
    xT = nc.dram_tensor("xT", [P, npcp], f32, kind="ExternalInput")
    mega = nc.dram_tensor("mega", [P, nblk * BW], i16, kind="ExternalInput")
    w1p = nc.dram_tensor("w1p", [P, 68], f32, kind="ExternalInput")
    w2t = nc.dram_tensor("w2t", [32, 128], f32, kind="ExternalInput")
    vu2 = nc.dram_tensor("vu2", [32, 8], f32, kind="ExternalInput")
    w3p = nc.dram_tensor("w3p", [P, 132], f32, kind="ExternalInput")
    as3r = nc.dram_tensor("as3r", [P, 128], bf16, kind="ExternalInput")
    b1r = nc.dram_tensor("b1r", [P, 32], f32, kind="ExternalInput")
    b2r = nc.dram_tensor("b2r", [P, 128], f32, kind="ExternalInput")
    b3r = nc.dram_tensor("b3r", [P, 32], f32, kind="ExternalInput")
    iot = nc.dram_tensor("iot", [P, P], bf16, kind="ExternalInput")
    out = nc.dram_tensor("out", [npc, 32], f32, kind="ExternalOutput")
    rg = [list(range(NCORE))]

    with tile.TileContext(nc) as tc:
        with tc.tile_pool(name="dramp", bufs=1, space="DRAM") as dramp, \
                tc.tile_pool(name="constp", bufs=1) as constp:
            t1loc = dramp.tile([npc, R12], bf16)
            t1full = dramp.tile([n, R12], bf16, addr_space="Shared")
            t2loc = dramp.tile([npc, R12], bf16)
            t2full = dramp.tile([n, R12], bf16, addr_space="Shared")
            t3loc = dramp.tile([npc, R3], bf16)
            t3full = dramp.tile([n, R3], bf16, addr_space="Shared")
            ad1 = dramp.tile([npc, RAD], f32)
            ad2 = dramp.tile([npc, RAD], f32)
            ad3 = dramp.tile([npc, RAD], f32)

            def cload(name, shape, dt, src):
                t = constp.tile(shape, dt, name=name)
                nc.sync.dma_start(t, src)
                return t

            w1p_s = cload("w1p_s", [P, 68], f32, w1p[:, :])
            w2t_s = cload("w2t_s", [32, 128], f32, w2t[:, :])
            vu2_s = cload("vu2_s", [32, 8], f32, vu2[:, :])
            w3p_s = cload("w3p_s", [P, 132], f32, w3p[:, :])
            as3_s = cload("as3_s", [P, 128], bf16, as3r[:, :])
            b1r_s = cload("b1r_s", [P, 32], f32, b1r[:, :])
            b2r_s = cload("b2r_s", [P, 128], f32, b2r[:, :])
            b3r_s = cload("b3r_s", [P, 32], f32, b3r[:, :])
            iot_s = cload("iot_s", [P, P], bf16, iot[:, :])
            ident = constp.tile([P, P], f32)
            make_identity(nc, ident)
            mega_s = constp.tile([P, nblk * BW], i16)
            nc.sync.dma_start(mega_s, mega[:, :])

            # parity-packed views: row pair (2g, 2g+1) -> packed row g; the
            # even/odd views address each half at stride 2*elem
            def parity_views(t, r):
                v = t.rearrange("(a b) c -> a (b c)", b=2)
                return v[:, 0:r], v[:, r:2 * r]

            t1a, t1b = parity_views(t1full, R12)
            t2a, t2b = parity_views(t2full, R12)
            t3a, t3b = parity_views(t3full, R3)

            # ---- stage A: T1 rows = [x@W1 | x@V1 | 0...], ad1 = x@U1 ----
            with tc.tile_pool(name="sa", bufs=3) as sa, \
                    tc.tile_pool(name="pa", bufs=2, space="PSUM") as pa:
                for b in range(nblk):
                    xb = sa.tile([P, P], f32, tag="xb")
                    nc.sync.dma_start(xb, xT[:, b * P:(b + 1) * P])
                    hp = pa.tile([P, 68], f32, tag="hp")
                    nc.tensor.matmul(hp, xb, w1p_s, start=True, stop=True)
                    hs = sa.tile([P, 64], bf16, tag="hs")
                    nc.vector.memset(hs[:, 36:64], 0.0)
                    nc.scalar.copy(hs[:, 0:36], hp[:, 0:36])
                    ha4 = sa.tile([P, 4], f32, tag="ha4")
                    nc.scalar.copy(ha4, hp[:, 64:68])
                    rows = min(P, npc - b * P)
                    nc.sync.dma_start(
                        t1loc[b * P:b * P + rows, :], hs[0:rows, :])
                    nc.sync.dma_start(
                        ad1[b * P:b * P + rows, 0:4], ha4[0:rows, :])

            def allgather(loc, full):
                if use_collectives:
                    nc.gpsimd.collective_compute(
                        "AllGather", Alu.bypass, replica_groups=rg,
                        ins=[loc[:, :].opt()], outs=[full[:, :].opt()])
                else:
                    nc.sync.dma_start(full[0:npc, :], loc[:, :])

            allgather(t1loc, t1full)

            if debug_tables:
                dbg1 = nc.dram_tensor("dbg1", [n, R12], bf16, kind="ExternalOutput")
                dbgad = nc.dram_tensor("dbgad", [npc, 4], f32, kind="ExternalOutput")
                dbg2 = nc.dram_tensor("dbg2", [npc, R12], bf16, kind="ExternalOutput")
                dbg3 = nc.dram_tensor("dbg3", [npc, R3], bf16, kind="ExternalOutput")
                dbgG = nc.dram_tensor("dbgG", [P, ST * R12], bf16, kind="ExternalOutput")
                dbgAD = nc.dram_tensor("dbgAD", [P, ST * 4], f32, kind="ExternalOutput")
                dbgPT = nc.dram_tensor("dbgPT", [P, ST * 4], bf16, kind="ExternalOutput")
                dbgST = nc.dram_tensor("dbgST", [P, ST * P], bf16, kind="ExternalOutput")
                dbgZN = nc.dram_tensor("dbgZN", [P, 32], f32, kind="ExternalOutput")

            def elu(sp, x_ap, cols, tag):
                mn = sp.tile([P, cols], f32, tag=tag + "mn")
                nc.vector.tensor_scalar_min(mn, x_ap, 0.0)
                ex = sp.tile([P, cols], f32, tag=tag + "ex")
                nc.scalar.activation(ex, mn, AF.Exp)
                mx = sp.tile([P, cols], f32, tag=tag + "mx")
                nc.vector.tensor_scalar_max(mx, x_ap, 0.0)
                sm = sp.tile([P, cols], f32, tag=tag + "sm")
                nc.vector.tensor_tensor(out=sm, in0=mx, in1=ex, op=Alu.add)
                res = sp.tile([P, cols], f32, tag=tag + "rs")
                nc.vector.tensor_scalar_add(res, sm, -1.0)
                return res

            def gat_layer(layer, tha, thb, adt, rpg, epilogue):
                """layer: 1/2/3. rpg: PG width (32 L1, 128 L2/L3)."""
                l3 = layer == 3
                Rt = R3 if l3 else R12
                gdt = bf16
                with tc.tile_pool(name=f"sp{layer}", bufs=4) as sp, \
                        tc.tile_pool(name=f"pp{layer}", bufs=2, space="PSUM") as pp:
                    for b in range(nblk):
                        base = b * BW
                        ixa = mega_s[:, base:base + WA]
                        ixb = mega_s[:, base + WA:base + WA + WB]
                        ixd = mega_s[:, base + WA + WB:base + WA + WB + WT]
                        relf = mega_s[:, base + WA + WB + WT:
                                      base + BW].bitcast(bf16)

                        G = sp.tile([P, ST * Rt], gdt, tag="G")
                        g3 = G.rearrange("p (s r) -> p s r", r=Rt)
                        _gather_calls(nc.gpsimd, g3, tha, thb,
                                      ixa, ixb, SA, SB, Rt, queue=0)
                        AD = sp.tile([P, ST * 4], f32, tag="AD")
                        adc = (ST + 3) // 4
                        c0 = 0
                        for j in range(4):
                            cl = min(adc, ST - c0)
                            if cl <= 0:
                                continue
                            _dma_gather_raw(
                                nc.gpsimd,
                                AD.rearrange("p (s h) -> p s h", h=4)[
                                    :, c0:c0 + cl, :],
                                adt, ixd[:, c0 * 8:(c0 + cl) * 8],
                                cl * P, 4, RAD, queue_num=1 + (j + 1) % 3)
                            c0 += cl

                        # e = leaky(asrc + adst); p = exp(e)
                        ee = sp.tile([P, ST * 4], f32, tag="ee")
                        if l3:
                            # asrc3 per edge = per-head dot(h3_row, as3)
                            gm = sp.tile([P, ST * 128], bf16, tag="gm")
                            nc.vector.tensor_tensor(
                                out=gm.rearrange("p (s h c) -> p s h c", h=4, c=32),
                                in0=g3.rearrange("p s (h c) -> p s h c", c=32),
                                in1=as3_s.rearrange(
                                    "p (h c) -> p h c", c=32).unsqueeze(1)
                                .broadcast_to([P, ST, 4, 32]),
                                op=Alu.mult)
                            ar = sp.tile([P, ST * 4], f32, tag="ar")
                            nc.vector.tensor_reduce(
                                out=ar.rearrange("p (s h) -> p s h", h=4),
                                in_=gm.rearrange("p (s h c) -> p s h c", h=4, c=32),
                                axis=mybir.AxisListType.X,
                                op=Alu.add)
                            nc.vector.tensor_tensor(
                                out=ee, in0=ar, in1=AD, op=Alu.add)
                        else:
                            nc.vector.tensor_tensor(
                                out=ee.rearrange("p (s h) -> p s h", h=4),
                                in0=g3[:, :, 32:36],
                                in1=AD.rearrange("p (s h) -> p s h", h=4),
                                op=Alu.add)
                        es = sp.tile([P, ST * 4], f32, tag="es")
                        nc.vector.tensor_scalar_mul(es, ee, NEG)
                        el = sp.tile([P, ST * 4], f32, tag="el")
                        nc.vector.tensor_tensor(out=el, in0=ee, in1=es, op=Alu.max)
                        pt = sp.tile([P, ST * 4], bf16, tag="pt")
                        nc.scalar.activation(pt, el, AF.Exp)

                        # S^T[p, s, j] = (dst_rel == j), bf16
                        st = sp.tile([P, ST * P], bf16, tag="st")
                        nc.vector.tensor_tensor(
                            out=st.rearrange("p (s j) -> p s j", j=P),
                            in0=relf.unsqueeze(2).broadcast_to([P, ST, P]),
                            in1=iot_s.unsqueeze(1).broadcast_to([P, ST, P]),
                            op=Alu.is_equal)

                        # rhs = [p*G-slices | p] bf16 per subtile
                        rw = rpg + 4
                        pgp = sp.tile([P, ST * rw], bf16, tag="pgp")
                        pg4 = pgp.rearrange("p (s m) -> p s m", m=rw)
                        pt3 = pt.rearrange("p (s h) -> p s h", h=4)
                        cw = rpg // 4
                        if layer == 1:
                            in0 = g3[:, :, 0:32].rearrange(
                                "p s (h c) -> p s h c", c=8)
                        elif layer == 2:
                            in0 = g3[:, :, 0:32].unsqueeze(2).broadcast_to(
                                [P, ST, 4, 32])
                        else:
                            in0 = g3.rearrange("p s (h c) -> p s h c", c=32)
                        nc.vector.tensor_tensor(
                            out=pg4[:, :, 0:rpg].rearrange(
                                "p s (h c) -> p s h c", c=cw),
                            in0=in0,
                            in1=pt3.unsqueeze(3).broadcast_to([P, ST, 4, cw]),
                            op=Alu.mult)
                        nc.vector.tensor_copy(out=pg4[:, :, rpg:rw], in_=pt3)

                        zb = pp.tile([P, rw], f32, tag="zb")
                        st3 = st.rearrange("p (s j) -> p s j", j=P)
                        for s in range(ST):
                            nc.tensor.matmul(
                                zb, st3[:, s, :], pg4[:, s, :],
                                start=(s == 0), stop=(s == ST - 1))

                        # znorm = z / (s + eps)
                        rr = sp.tile([P, 4], f32, tag="rr")
                        nc.vector.tensor_scalar_add(rr, zb[:, rpg:rw], EPS)
                        rr2 = sp.tile([P, 4], f32, tag="rr2")
                        nc.vector.reciprocal(rr2, rr)
                        zn = sp.tile([P, rpg], f32, tag="zn")
                        nc.vector.tensor_tensor(
                            out=zn.rearrange("p (h c) -> p h c", c=cw),
                            in0=zb[:, 0:rpg].rearrange("p (h c) -> p h c", c=cw),
                            in1=rr2.unsqueeze(2).broadcast_to([P, 4, cw]),
                            op=Alu.mult)

                        if debug_tables and layer == 1 and b == 0:
                            nc.sync.dma_start(dbgG[:, :], G)
                            nc.sync.dma_start(dbgAD[:, :], AD)
                            nc.sync.dma_start(dbgPT[:, :], pt)
                            nc.sync.dma_start(dbgST[:, :], st)
                            nc.sync.dma_start(dbgZN[:, :], zn)

                        epilogue(b, sp, pp, zn)

            # ---- layer epilogues ----
            def epi1(b, sp, pp, zn):
                tb = sp.tile([P, 32], f32, tag="tb")
                nc.vector.tensor_tensor(out=tb, in0=zn, in1=b1r_s, op=Alu.add)
                t2 = elu(sp, tb, 32, "e1")
                t2T_p = pp.tile([32, P], f32, tag="t2T_p", bufs=1)
                nc.tensor.transpose(t2T_p, t2, ident)
                t2T = sp.tile([32, P], f32, tag="t2T")
                nc.scalar.copy(t2T, t2T_p)
                av_p = pp.tile([P, 8], f32, tag="av_p", bufs=1)
                nc.tensor.matmul(av_p, t2T, vu2_s, start=True, stop=True)
                stg = sp.tile([P, 64], bf16, tag="stg")
                nc.vector.memset(stg[:, 36:64], 0.0)
                nc.scalar.copy(stg[:, 0:32], t2)
                av = sp.tile([P, 8], f32, tag="av")
                nc.scalar.copy(av, av_p)
                nc.vector.tensor_copy(out=stg[:, 32:36], in_=av[:, 0:4])
                rows = min(P, npc - b * P)
                nc.sync.dma_start(t2loc[b * P:b * P + rows, :], stg[0:rows, :])
                nc.sync.dma_start(ad2[b * P:b * P + rows, 0:4], av[0:rows, 4:8])

            def epi2(b, sp, pp, zn):
                o2 = pp.tile([P, 128], f32, tag="o2", bufs=1)
                for h in range(4):
                    zT_p = pp.tile([32, P], f32, tag="zT_p", bufs=2)
                    nc.tensor.transpose(zT_p, zn[:, 32 * h:32 * h + 32], ident)
                    zT = sp.tile([32, P], f32, tag="zT")
                    nc.scalar.copy(zT, zT_p)
                    nc.tensor.matmul(
                        o2[:, 32 * h:32 * h + 32], zT,
                        w2t_s[:, 32 * h:32 * h + 32], start=True, stop=True)
                ob = sp.tile([P, 128], f32, tag="ob")
                nc.vector.tensor_tensor(out=ob, in0=o2, in1=b2r_s, op=Alu.add)
                t3 = elu(sp, ob, 128, "e2")
                t3T_p = pp.tile([P, P], f32, tag="t3T_p", bufs=1)
                nc.tensor.transpose(t3T_p, t3, ident)
                t3T = sp.tile([P, P], f32, tag="t3T")
                nc.scalar.copy(t3T, t3T_p)
                hv_p = pp.tile([P, 132], f32, tag="hv_p", bufs=1)
                nc.tensor.matmul(hv_p, t3T, w3p_s, start=True, stop=True)
                hvb = sp.tile([P, 128], bf16, tag="hvb")
                nc.scalar.copy(hvb, hv_p[:, 0:128])
                hva = sp.tile([P, 4], f32, tag="hva")
                nc.scalar.copy(hva, hv_p[:, 128:132])
                rows = min(P, npc - b * P)
                nc.sync.dma_start(t3loc[b * P:b * P + rows, :], hvb[0:rows, :])
                nc.sync.dma_start(ad3[b * P:b * P + rows, 0:4], hva[0:rows, :])

            def epi3(b, sp, pp, zn):
                a1 = sp.tile([P, 32], f32, tag="a1")
                nc.vector.tensor_tensor(
                    out=a1, in0=zn[:, 0:32], in1=zn[:, 32:64], op=Alu.add)
                a2 = sp.tile([P, 32], f32, tag="a2")
                nc.vector.tensor_tensor(
                    out=a2, in0=a1, in1=zn[:, 64:96], op=Alu.add)
                a3 = sp.tile([P, 32], f32, tag="a3")
                nc.vector.tensor_tensor(
                    out=a3, in0=a2, in1=zn[:, 96:128], op=Alu.add)
                a4 = sp.tile([P, 32], f32, tag="a4")
                nc.vector.tensor_scalar_mul(a4, a3, 0.25)
                a5 = sp.tile([P, 32], f32, tag="a5")
                nc.vector.tensor_tensor(out=a5, in0=a4, in1=b3r_s, op=Alu.add)
                rows = min(P, npc - b * P)
                nc.sync.dma_start(out[b * P:b * P + rows, :], a5[0:rows, :])

            gat_layer(1, t1a, t1b, ad1[:, 0:4], 32, epi1)
            allgather(t2loc, t2full)
            gat_layer(2, t2a, t2b, ad2[:, 0:4], 128, epi2)
            allgather(t3loc, t3full)
            gat_layer(3, t3a, t3b, ad3[:, 0:4], 128, epi3)

            if debug_tables:
                nc.sync.dma_start(dbg1[:, :], t1full[:, :])
                nc.sync.dma_start(dbgad[:, :], ad1[:, 0:4])
                nc.sync.dma_start(dbg2[:, :], t2loc[:, :])
                nc.sync.dma_start(dbg3[:, :], t3loc[:, :])

    nc.compile()
    return nc


# ---------------------------------------------------------------- entry point

def _host_inputs(inputs, n):
    x = np.asarray(inputs["x"], np.float32)
    edge_index = np.asarray(inputs["edge_index"])
    W1 = np.asarray(inputs["W1"], np.float32)
    W2 = np.asarray(inputs["W2"], np.float32)
    W3 = np.asarray(inputs["W3"], np.float32)
    as1, ad1 = np.asarray(inputs["as1"]), np.asarray(inputs["ad1"])
    as2, ad2 = np.asarray(inputs["as2"]), np.asarray(inputs["ad2"])
    as3, ad3 = np.asarray(inputs["as3"]), np.asarray(inputs["ad3"])
    b1, b2, b3 = [np.asarray(inputs[k], np.float32) for k in ("b1", "b2", "b3")]

    gxa, gxb, adx, rl, SA, SB, npc, nblk = _preprocess(edge_index, n, NCORE)
    npcp = nblk * P

    # w1p: [W1(32) | V1(4) | zeros(28) | U1(4)] -> psum cols 0:64 = table row
    w1pack = np.concatenate(
        [W1, _make_vu(W1, as1, H, 8), np.zeros((IN_C, 28), np.float32),
         _make_vu(W1, ad1, H, 8)], axis=1)
    vu2p = np.concatenate(
        [_make_vu(W2, as2, H, 32), _make_vu(W2, ad2, H, 32)], axis=1)
    w3pack = np.concatenate([W3, _make_vu(W3, ad3, H, 32)], axis=1)
    as3rep = np.tile(np.asarray(as3, np.float32).reshape(1, 128), (P, 1))

    import ml_dtypes
    ST = SA + SB
    WA, WB, WT = SA * 8, max(SB, 1) * 8, ST * 8
    rl16 = rl.astype(ml_dtypes.bfloat16).view(np.int16)
    megas = np.concatenate([gxa, gxb, adx, rl16], axis=3)  # [nc, nblk, 128, BW]
    megas = megas.transpose(0, 2, 1, 3).reshape(NCORE, P, -1)
    iota = np.tile(np.arange(P, dtype=np.float32), (P, 1)).astype(ml_dtypes.bfloat16)
    reps = {
        "w1p": w1pack.astype(np.float32),
        "w2t": np.ascontiguousarray(W2),
        "vu2": np.ascontiguousarray(vu2p),
        "w3p": w3pack.astype(np.float32),
        "as3r": as3rep,  # cast to bf16 at dma? inputs must match dtype: cast below
        "b1r": np.tile(b1, (P, 1)).astype(np.float32),
        "b2r": np.tile(b2, (P, 1)).astype(np.float32),
        "b3r": np.tile(b3, (P, 1)).astype(np.float32),
        "iot": iota,
    }
    reps["as3r"] = as3rep.astype(ml_dtypes.bfloat16)

    in_maps = []
    for k in range(NCORE):
        xk = x[k * npc:(k + 1) * npc]
        xT = np.zeros((P, npcp), np.float32)
        xT[:, :npc] = xk.T
        m = {"xT": xT, "mega": np.ascontiguousarray(megas[k])}
        m.update(reps)
        in_maps.append(m)
    return in_maps, SA, SB, npc, nblk


def _run(inputs, n, e, trace=False):
    from concourse.bass_utils import run_bass_kernel_spmd

    in_maps, SA, SB, npc, nblk = _host_inputs(inputs, n)
    nc = _build_program(n, SA, SB, npc, nblk)
    res = run_bass_kernel_spmd(
        nc, in_maps, core_ids=list(range(NCORE)), trace=trace)
    outs = [res.results[k]["out"] for k in range(NCORE)]
    full = np.concatenate(outs, axis=0).astype(np.float32)
    return full, res


def kernel(**inputs):
    full, _ = _run(inputs, N, E, trace=False)
    return full



# revision 6
# speedup vs baseline: 1.6606x; 1.4740x over previous
"""Trainium2 Bass kernel for a 3-layer GAT (nn_GATModel_32229434589362).

Strategy (dst-sharded, edge-major S-matrix aggregation, Ant dma_gather):
  - Nodes sharded by destination across 8 cores (6250/core); each core owns
    all edges with dst in its range (host-side bucketing of edge_index only).
  - Per layer a per-node gather table holds [per-head features | a_src]
    (L1/L2: 64xbf16 rows, 256B pair stride; L3: [h3 128 | asrc3 4] in
    192xbf16 rows, 768B pair stride), AllGather-replicated.
  - Gathers use InstDMAGatherAnt (int16 idx). The 50000-row range is covered
    by splitting each block's edges into src-even / src-odd groups gathered
    from parity-packed views of the same table (idx = src>>1 < 25000).
  - a_dst per edge is NOT gathered: a one-hot stj[j, (s p)] = (dst_rel==j)
    is precomputed on host (graph static, shared by all 3 layers) and the
    per-edge a_dst expands as per-subtile PE matmuls stj_s^T @ adst_blk
    into PSUM. adst tables are bf16 [nblk*128, 4] (block-padded).
  - p = exp(leaky(asrc+adst)) with NO max subtraction (scores are O(0.5);
    softmax is shift-invariant); normalization deferred: out = z/(s+1e-16).
  - Segment reduction per 128-node block: z,s accumulate over subtiles as PE
    matmuls with lhsT = S^T (bf16 indicator built by one is_equal) and
    rhs = [p*G | p].
"""
import sys

sys.path.insert(0, "/opt/trn_rl_repo")

import numpy as np

import concourse.bass as bass
import concourse.bacc as bacc
import concourse.tile as tile
import concourse.mybir as mybir
from concourse.masks import make_identity

f32 = mybir.dt.float32
bf16 = mybir.dt.bfloat16
i16 = mybir.dt.int16
AF = mybir.ActivationFunctionType
Alu = mybir.AluOpType

# problem constants
N, E, IN_C, H = 50000, 800000, 128, 4
NCORE, P = 8, 128
NEG = 0.2
EPS = 1e-16
R12 = 64        # T1/T2 row: [feat 32 | asrc 4 | zeros] bf16 (128B)
R3 = 192        # T3 row: [h3 128 | asrc3 4 | pad] bf16 (384B, pair stride 768B)
R3V = 132       # T3 gathered payload elems
PAD_REL = 200.0


# ---------------------------------------------------------------- host side

def _make_vu(W, att, heads, c):
    return np.stack(
        [W[:, h * c:(h + 1) * c] @ att[h] for h in range(heads)], axis=1
    ).astype(np.float32)


def _wrap16(vals):
    """int16 idx list -> [128, ceil(n/16)] wrapped layout: idx i at
    [i%16, i//16], replicated down all 8 groups of 16 partitions."""
    n = vals.shape[0]
    w = -(-n // 16)
    a = np.zeros((16, w), np.int16)
    a[(np.arange(n) % 16), (np.arange(n) // 16)] = vals.astype(np.int16)
    return np.tile(a, (8, 1))


def _preprocess(edge_index, n, ncore):
    """Bucket edges by (dst core, dst block), split each bucket by src parity.

    Slot (p, s) of a block = edge list position s*128+p. Group A (src even)
    occupies subtiles [0, SA), group B (odd) [SA, SA+SB). Gather idx is
    src>>1 (< 25000, fits int16); table rows are parity-packed in pairs.
      gxa [nblk, 128, SA*8] int16  (gather idx, wrapped layout, pad=0)
      gxb [nblk, 128, SB*8] int16
      rel [nblk, 128, ST] f32      (dst_rel, pad=PAD_REL)
      stj [nblk, 128, ST*128] bf16 (one-hot: stj[j, s*128+p] = rel[p,s]==j)
    """
    import ml_dtypes
    npc = n // ncore
    nblk = (npc + P - 1) // P
    src = np.asarray(edge_index[0], np.int64)
    dst = np.asarray(edge_index[1], np.int64)
    core = dst // npc
    dloc = dst % npc
    blk = dloc // P
    rel = (dloc % P).astype(np.float32)
    grp = (src & 1).astype(np.int64)

    nb = ncore * nblk
    key = (core * nblk + blk) * 2 + grp
    counts = np.bincount(key, minlength=nb * 2).reshape(nb, 2)
    SA = int(-(-counts[:, 0].max() // P))
    SB = int(-(-counts[:, 1].max() // P))
    ST = SA + SB

    # slot position within the block for each edge
    order = np.argsort(key, kind="stable")
    ksort = key[order]
    starts = np.zeros(nb * 2 + 1, np.int64)
    np.cumsum(counts.reshape(-1), out=starts[1:])
    slot_in_grp = np.arange(src.shape[0]) - starts[ksort]
    bucket = ksort // 2
    grp_s = ksort % 2
    pos = np.where(grp_s == 0, slot_in_grp, SA * P + slot_in_grp)
    flat = bucket * (ST * P) + pos

    gx = np.zeros(nb * ST * P, np.int64)
    rl = np.full(nb * ST * P, PAD_REL, np.float32)
    sv = src[order]
    gx[flat] = sv >> 1
    rl[flat] = rel[order]

    gx = gx.reshape(ncore, nblk, ST * P)
    rl = rl.reshape(ncore, nblk, ST, P)      # [c, b, s, p]

    gxa = np.zeros((ncore, nblk, 128, SA * 8), np.int16)
    gxb = np.zeros((ncore, nblk, 128, max(SB, 1) * 8), np.int16)
    for k in range(ncore):
        for b in range(nblk):
            gxa[k, b] = _wrap16(gx[k, b, :SA * P])
            if SB:
                gxb[k, b] = _wrap16(gx[k, b, SA * P:])

    # stj one-hot [c, b, j, (s p)]: edge at slot (p, s) with dst_rel j
    stj = np.zeros((ncore, nblk, P, ST * P), ml_dtypes.bfloat16)
    ci, bi = np.divmod(np.arange(nb)[bucket], nblk)
    pp = (pos % P).astype(np.int64)
    ss = (pos // P).astype(np.int64)
    jj = rel[order].astype(np.int64)
    stj[ci, bi, jj, ss * P + pp] = 1.0

    # rel as [c, b, p, s] (partition-major slots) for the device-side st
    rl = np.ascontiguousarray(rl.transpose(0, 1, 3, 2))
    return gxa, gxb, rl, stj, SA, SB, npc, nblk


# ---------------------------------------------------------------- device side

def _gather_calls(gp, out3, table_even, table_odd, idxa, idxb,
                  SA, SB, elem, elem_step):
    """2 calls per parity group (ceil/floor split, <=1024 descs each).
    Queues 1/2 (async posts), balanced so each queue gets c1+rest descs.
    out3: [128, ST, elem]."""
    qmap = (1, 2, 2, 1)
    pos = 0
    for base, cnt, tbl, idx in ((0, SA, table_even, idxa),
                                (SA, SB, table_odd, idxb)):
        c1 = (cnt + 1) // 2
        for c0, cl in ((0, c1), (c1, cnt - c1)):
            if cl <= 0:
                pos += 1
                continue
            assert cl * P <= 1024, "SWDGE ring limit"
            num = cl * P
            _dma_gather_raw(
                gp,
                out3[:, base + c0:base + c0 + cl, :],
                tbl,
                idx[:, c0 * 8:(c0 + cl) * 8],
                num, elem, elem_step,
                queue_num=qmap[pos],
            )
            pos += 1


def _dma_gather_raw(gp, out_ap, in_ap, idxs_ap, num_idxs, elem_size,
                    elem_step, queue_num=0):
    """dma_gather (DRAM, non-transpose) minus the elem_size%256 assert — the
    Q7 ucode is size-agnostic here; only the stride must be 256B-aligned."""
    from concourse import ap_utils
    assert idxs_ap.dtype == mybir.dt.int16
    assert in_ap.dtype == out_ap.dtype
    assert ap_utils.ap_is_contiguous(out_ap.ap[1:])
    assert ap_utils.ap_is_contiguous(idxs_ap.ap[1:])
    assert in_ap.ap[0][0] == elem_step and in_ap.ap[-1][1] == elem_size
    assert out_ap.ap[-1][1] == elem_size
    assert out_ap.ap[0][1] * out_ap.ap[1][1] == num_idxs
    stride_bytes_256 = (elem_step * mybir.dt.size(in_ap.dtype)) // 256
    assert stride_bytes_256 * 256 == elem_step * mybir.dt.size(in_ap.dtype)
    _in_ap = gp.lower_ap_dma(in_ap, for_custom_bir_dma=True)
    inst = gp.add_instruction(
        mybir.InstDMAGatherAnt(
            name=gp.bass.get_next_instruction_name(),
            ins=[*_in_ap, gp.lower_ap(idxs_ap),
                 gp.lower_val_access(gp.to_reg(num_idxs))],
            outs=[gp.lower_ap(out_ap)],
            transpose=False,
            num_idxs=num_idxs,
            elem_size=elem_size,
            stride_bytes_256=stride_bytes_256,
            gen_mode=0,
            single_packet=True,
            queue_num=queue_num,
            sbuf_tokens_per_rank=0,
            sbuf_free_dim_per_rank=0,
            sbuf_free_dim_pad_per_rank=0,
            sbuf_byte_offset=0,
        )
    )
    return inst


def _build_program(n, SA, SB, npc, nblk, use_collectives=True):
    nc = bacc.Bacc("TRN2", num_devices=NCORE, num_swdge_queues=4)
    ST = SA + SB
    npcp = nblk * P
    WA, WB = SA * 8, max(SB, 1) * 8

    BW = WA + WB + ST        # int16 cols per block in the mega index tile
    xT = nc.dram_tensor("xT", [P, npcp], f32, kind="ExternalInput")
    mega = nc.dram_tensor("mega", [P, nblk * BW], i16, kind="ExternalInput")
    stjt = nc.dram_tensor("stjt", [P, nblk * ST * P], bf16,
                          kind="ExternalInput")
    w1p = nc.dram_tensor("w1p", [P, 68], f32, kind="ExternalInput")
    w2t = nc.dram_tensor("w2t", [32, 128], f32, kind="ExternalInput")
    vu2 = nc.dram_tensor("vu2", [32, 8], f32, kind="ExternalInput")
    w3p = nc.dram_tensor("w3p", [P, 136], f32, kind="ExternalInput")
    b1r = nc.dram_tensor("b1r", [P, 32], f32, kind="ExternalInput")
    b2r = nc.dram_tensor("b2r", [P, 128], f32, kind="ExternalInput")
    b3r = nc.dram_tensor("b3r", [P, 32], f32, kind="ExternalInput")
    iot = nc.dram_tensor("iot", [P, P], bf16, kind="ExternalInput")
    out = nc.dram_tensor("out", [npc, 32], f32, kind="ExternalOutput")
    rg = [list(range(NCORE))]

    with tile.TileContext(nc) as tc:
        with tc.tile_pool(name="dramp", bufs=1, space="DRAM") as dramp, \
                tc.tile_pool(name="constp", bufs=1) as constp:
            t1loc = dramp.tile([npc, R12], bf16)
            t1full = dramp.tile([n, R12], bf16, addr_space="Shared")
            t2loc = dramp.tile([npc, R12], bf16)
            t2full = dramp.tile([n, R12], bf16, addr_space="Shared")
            t3loc = dramp.tile([npc, R3], bf16)
            t3full = dramp.tile([n, R3], bf16, addr_space="Shared")
            ad1 = dramp.tile([npcp, 4], bf16)
            ad2 = dramp.tile([npcp, 4], bf16)
            ad3 = dramp.tile([npcp, 4], bf16)

            def cload(name, shape, dt, src):
                t = constp.tile(shape, dt, name=name)
                nc.sync.dma_start(t, src)
                return t

            w1p_s = cload("w1p_s", [P, 68], f32, w1p[:, :])
            w2t_s = cload("w2t_s", [32, 128], f32, w2t[:, :])
            vu2_s = cload("vu2_s", [32, 8], f32, vu2[:, :])
            w3p_s = cload("w3p_s", [P, 136], f32, w3p[:, :])
            b1r_s = cload("b1r_s", [P, 32], f32, b1r[:, :])
            b2r_s = cload("b2r_s", [P, 128], f32, b2r[:, :])
            b3r_s = cload("b3r_s", [P, 32], f32, b3r[:, :])
            iot_s = cload("iot_s", [P, P], bf16, iot[:, :])
            ident = constp.tile([P, P], f32)
            make_identity(nc, ident)
            mega_s = constp.tile([P, nblk * BW], i16)
            nc.sync.dma_start(mega_s, mega[:, :])

            # parity-packed views: row pair (2g, 2g+1) -> packed row g; the
            # even/odd views address each half at stride 2*r elems
            def parity_views(t, r, v):
                w = t.rearrange("(a b) c -> a (b c)", b=2)
                return w[:, 0:v], w[:, r:r + v]

            t1a, t1b = parity_views(t1full, R12, R12)
            t2a, t2b = parity_views(t2full, R12, R12)
            t3a, t3b = parity_views(t3full, R3, R3V)

            # ---- stage A: T1 rows = [x@W1 | x@V1 | 0...], ad1 = x@U1 ----
            with tc.tile_pool(name="sa", bufs=3) as sa, \
                    tc.tile_pool(name="pa", bufs=2, space="PSUM") as pa:
                for b in range(nblk):
                    xb = sa.tile([P, P], f32, tag="xb")
                    nc.sync.dma_start(xb, xT[:, b * P:(b + 1) * P])
                    hp = pa.tile([P, 68], f32, tag="hp")
                    nc.tensor.matmul(hp, xb, w1p_s, start=True, stop=True)
                    hs = sa.tile([P, 64], bf16, tag="hs")
                    nc.vector.memset(hs[:, 36:64], 0.0)
                    nc.scalar.copy(hs[:, 0:36], hp[:, 0:36])
                    ha4 = sa.tile([P, 4], bf16, tag="ha4")
                    nc.scalar.copy(ha4, hp[:, 64:68])
                    rows = min(P, npc - b * P)
                    nc.sync.dma_start(
                        t1loc[b * P:b * P + rows, :], hs[0:rows, :])
                    nc.sync.dma_start(ad1[b * P:(b + 1) * P, :], ha4)

            def allgather(loc, full):
                if use_collectives:
                    nc.gpsimd.collective_compute(
                        "AllGather", Alu.bypass, replica_groups=rg,
                        ins=[loc[:, :].opt()], outs=[full[:, :].opt()])
                else:
                    nc.sync.dma_start(full[0:npc, :], loc[:, :])

            allgather(t1loc, t1full)

            def elu(sp, x_ap, cols, tag):
                mn = sp.tile([P, cols], f32, tag=tag + "mn")
                nc.vector.tensor_scalar_min(mn, x_ap, 0.0)
                ex = sp.tile([P, cols], f32, tag=tag + "ex")
                nc.scalar.activation(ex, mn, AF.Exp)
                mx = sp.tile([P, cols], f32, tag=tag + "mx")
                nc.vector.tensor_scalar_max(mx, x_ap, 0.0)
                sm = sp.tile([P, cols], f32, tag=tag + "sm")
                nc.vector.tensor_tensor(out=sm, in0=mx, in1=ex, op=Alu.add)
                res = sp.tile([P, cols], f32, tag=tag + "rs")
                nc.vector.tensor_scalar_add(res, sm, -1.0)
                return res

            def gat_layer(layer, tha, thb, adt, rpg, epilogue):
                """layer: 1/2/3. rpg: PG width (32 L1, 128 L2/L3)."""
                l3 = layer == 3
                Rt = R3V if l3 else R12
                estep = 2 * R3 if l3 else 2 * R12
                with tc.tile_pool(name=f"sp{layer}", bufs=4) as sp, \
                        tc.tile_pool(name=f"pp{layer}", bufs=2, space="PSUM") as pp:
                    for b in range(nblk):
                        base = b * BW
                        ixa = mega_s[:, base:base + WA]
                        ixb = mega_s[:, base + WA:base + WA + WB]
                        relf = mega_s[:, base + WA + WB:
                                      base + BW].bitcast(bf16)

                        G = sp.tile([P, ST * Rt], bf16, tag="G")
                        g3 = G.rearrange("p (s r) -> p s r", r=Rt)
                        _gather_calls(nc.gpsimd, g3, tha, thb,
                                      ixa, ixb, SA, SB, Rt, estep)

                        # stj one-hot [j, (s p)] + adst block tile
                        SJ = sp.tile([P, ST * P], bf16, tag="SJ")
                        nc.sync.dma_start(
                            SJ, stjt[:, b * ST * P:(b + 1) * ST * P])
                        sj3 = SJ.rearrange("p (s q) -> p s q", q=P)
                        adb = sp.tile([P, 4], bf16, tag="adb")
                        nc.sync.dma_start(adb, adt[b * P:(b + 1) * P, :])

                        # ADe[p, s, h] = sum_j stj[j, (s p)] * adst[j, h]
                        eep = pp.tile([P, ST * 4], f32, tag="eep", bufs=1)
                        for s in range(ST):
                            nc.tensor.matmul(
                                eep[:, 4 * s:4 * s + 4], sj3[:, s, :], adb,
                                start=True, stop=True)

                        # e = leaky(asrc + adst); p = exp(e)
                        ee = sp.tile([P, ST * 4], f32, tag="ee")
                        a0 = 128 if l3 else 32
                        nc.vector.tensor_tensor(
                            out=ee.rearrange("p (s h) -> p s h", h=4),
                            in0=g3[:, :, a0:a0 + 4],
                            in1=eep.rearrange("p (s h) -> p s h", h=4),
                            op=Alu.add)
                        es = sp.tile([P, ST * 4], f32, tag="es")
                        nc.vector.tensor_scalar_mul(es, ee, NEG)
                        el = sp.tile([P, ST * 4], f32, tag="el")
                        nc.vector.tensor_tensor(out=el, in0=ee, in1=es, op=Alu.max)
                        pt = sp.tile([P, ST * 4], bf16, tag="pt")
                        nc.scalar.activation(pt, el, AF.Exp)

                        # S^T[p, s, j] = (dst_rel == j), bf16
                        st = sp.tile([P, ST * P], bf16, tag="st")
                        nc.vector.tensor_tensor(
                            out=st.rearrange("p (s j) -> p s j", j=P),
                            in0=relf.unsqueeze(2).broadcast_to([P, ST, P]),
                            in1=iot_s.unsqueeze(1).broadcast_to([P, ST, P]),
                            op=Alu.is_equal)

                        # rhs = [p*G-slices | p] bf16 per subtile
                        rw = rpg + 4
                        pgp = sp.tile([P, ST * rw], bf16, tag="pgp")
                        pg4 = pgp.rearrange("p (s m) -> p s m", m=rw)
                        pt3 = pt.rearrange("p (s h) -> p s h", h=4)
                        cw = rpg // 4
                        if layer == 1:
                            in0 = g3[:, :, 0:32].rearrange(
                                "p s (h c) -> p s h c", c=8)
                        elif layer == 2:
                            in0 = g3[:, :, 0:32].unsqueeze(2).broadcast_to(
                                [P, ST, 4, 32])
                        else:
                            in0 = g3[:, :, 0:128].rearrange(
                                "p s (h c) -> p s h c", c=32)
                        nc.vector.tensor_tensor(
                            out=pg4[:, :, 0:rpg].rearrange(
                                "p s (h c) -> p s h c", c=cw),
                            in0=in0,
                            in1=pt3.unsqueeze(3).broadcast_to([P, ST, 4, cw]),
                            op=Alu.mult)
                        nc.vector.tensor_copy(out=pg4[:, :, rpg:rw], in_=pt3)

                        zb = pp.tile([P, rw], f32, tag="zb")
                        st3 = st.rearrange("p (s j) -> p s j", j=P)
                        for s in range(ST):
                            nc.tensor.matmul(
                                zb, st3[:, s, :], pg4[:, s, :],
                                start=(s == 0), stop=(s == ST - 1))

                        # znorm = z / (s + eps)
                        rr = sp.tile([P, 4], f32, tag="rr")
                        nc.vector.tensor_scalar_add(rr, zb[:, rpg:rw], EPS)
                        rr2 = sp.tile([P, 4], f32, tag="rr2")
                        nc.vector.reciprocal(rr2, rr)
                        zn = sp.tile([P, rpg], f32, tag="zn")
                        nc.vector.tensor_tensor(
                            out=zn.rearrange("p (h c) -> p h c", c=cw),
                            in0=zb[:, 0:rpg].rearrange("p (h c) -> p h c", c=cw),
                            in1=rr2.unsqueeze(2).broadcast_to([P, 4, cw]),
                            op=Alu.mult)

                        epilogue(b, sp, pp, zn)

            # ---- layer epilogues ----
            def epi1(b, sp, pp, zn):
                tb = sp.tile([P, 32], f32, tag="tb")
                nc.vector.tensor_tensor(out=tb, in0=zn, in1=b1r_s, op=Alu.add)
                t2 = elu(sp, tb, 32, "e1")
                t2T_p = pp.tile([32, P], f32, tag="t2T_p", bufs=1)
                nc.tensor.transpose(t2T_p, t2, ident)
                t2T = sp.tile([32, P], f32, tag="t2T")
                nc.scalar.copy(t2T, t2T_p)
                av_p = pp.tile([P, 8], f32, tag="av_p", bufs=1)
                nc.tensor.matmul(av_p, t2T, vu2_s, start=True, stop=True)
                stg = sp.tile([P, 64], bf16, tag="stg")
                nc.vector.memset(stg[:, 36:64], 0.0)
                nc.scalar.copy(stg[:, 0:32], t2)
                nc.scalar.copy(stg[:, 32:36], av_p[:, 0:4])
                avb = sp.tile([P, 4], bf16, tag="avb")
                nc.scalar.copy(avb, av_p[:, 4:8])
                rows = min(P, npc - b * P)
                nc.sync.dma_start(t2loc[b * P:b * P + rows, :], stg[0:rows, :])
                nc.sync.dma_start(ad2[b * P:(b + 1) * P, :], avb)

            def epi2(b, sp, pp, zn):
                o2 = pp.tile([P, 128], f32, tag="o2", bufs=1)
                for h in range(4):
                    zT_p = pp.tile([32, P], f32, tag="zT_p", bufs=2)
                    nc.tensor.transpose(zT_p, zn[:, 32 * h:32 * h + 32], ident)
                    zT = sp.tile([32, P], f32, tag="zT")
                    nc.scalar.copy(zT, zT_p)
                    nc.tensor.matmul(
                        o2[:, 32 * h:32 * h + 32], zT,
                        w2t_s[:, 32 * h:32 * h + 32], start=True, stop=True)
                ob = sp.tile([P, 128], f32, tag="ob")
                nc.vector.tensor_tensor(out=ob, in0=o2, in1=b2r_s, op=Alu.add)
                t3 = elu(sp, ob, 128, "e2")
                t3T_p = pp.tile([P, P], f32, tag="t3T_p", bufs=1)
                nc.tensor.transpose(t3T_p, t3, ident)
                t3T = sp.tile([P, P], f32, tag="t3T")
                nc.scalar.copy(t3T, t3T_p)
                hv_p = pp.tile([P, 136], f32, tag="hv_p", bufs=1)
                nc.tensor.matmul(hv_p, t3T, w3p_s, start=True, stop=True)
                hvb = sp.tile([P, R3V], bf16, tag="hvb")
                nc.scalar.copy(hvb, hv_p[:, 0:R3V])
                hva = sp.tile([P, 4], bf16, tag="hva")
                nc.scalar.copy(hva, hv_p[:, 132:136])
                rows = min(P, npc - b * P)
                nc.sync.dma_start(
                    t3loc[b * P:b * P + rows, 0:R3V], hvb[0:rows, :])
                nc.sync.dma_start(ad3[b * P:(b + 1) * P, :], hva)

            def epi3(b, sp, pp, zn):
                a1 = sp.tile([P, 32], f32, tag="a1")
                nc.vector.tensor_tensor(
                    out=a1, in0=zn[:, 0:32], in1=zn[:, 32:64], op=Alu.add)
                a2 = sp.tile([P, 32], f32, tag="a2")
                nc.vector.tensor_tensor(
                    out=a2, in0=a1, in1=zn[:, 64:96], op=Alu.add)
                a3 = sp.tile([P, 32], f32, tag="a3")
                nc.vector.tensor_tensor(
                    out=a3, in0=a2, in1=zn[:, 96:128], op=Alu.add)
                a4 = sp.tile([P, 32], f32, tag="a4")
                nc.vector.tensor_scalar_mul(a4, a3, 0.25)
                a5 = sp.tile([P, 32], f32, tag="a5")
                nc.vector.tensor_tensor(out=a5, in0=a4, in1=b3r_s, op=Alu.add)
                rows = min(P, npc - b * P)
                nc.sync.dma_start(out[b * P:b * P + rows, :], a5[0:rows, :])

            gat_layer(1, t1a, t1b, ad1, 32, epi1)
            allgather(t2loc, t2full)
            gat_layer(2, t2a, t2b, ad2, 128, epi2)
            allgather(t3loc, t3full)
            gat_layer(3, t3a, t3b, ad3, 128, epi3)

    nc.compile()
    return nc


# ---------------------------------------------------------------- entry point

def _host_inputs(inputs, n):
    x = np.asarray(inputs["x"], np.float32)
    edge_index = np.asarray(inputs["edge_index"])
    W1 = np.asarray(inputs["W1"], np.float32)
    W2 = np.asarray(inputs["W2"], np.float32)
    W3 = np.asarray(inputs["W3"], np.float32)
    as1, ad1 = np.asarray(inputs["as1"]), np.asarray(inputs["ad1"])
    as2, ad2 = np.asarray(inputs["as2"]), np.asarray(inputs["ad2"])
    as3, ad3 = np.asarray(inputs["as3"]), np.asarray(inputs["ad3"])
    b1, b2, b3 = [np.asarray(inputs[k], np.float32) for k in ("b1", "b2", "b3")]

    gxa, gxb, rl, stj, SA, SB, npc, nblk = _preprocess(edge_index, n, NCORE)
    npcp = nblk * P

    # w1p: [W1(32) | V1(4) | zeros(28) | U1(4)] -> psum cols 0:64 = table row
    w1pack = np.concatenate(
        [W1, _make_vu(W1, as1, H, 8), np.zeros((IN_C, 28), np.float32),
         _make_vu(W1, ad1, H, 8)], axis=1)
    vu2p = np.concatenate(
        [_make_vu(W2, as2, H, 32), _make_vu(W2, ad2, H, 32)], axis=1)
    # w3p: [W3 (128) | V3=as3 proj (4) | U3=ad3 proj (4)]
    w3pack = np.concatenate(
        [W3, _make_vu(W3, as3, H, 32), _make_vu(W3, ad3, H, 32)], axis=1)

    import ml_dtypes
    ST = SA + SB
    WA, WB = SA * 8, max(SB, 1) * 8
    rl16 = rl.astype(ml_dtypes.bfloat16).view(np.int16)
    megas = np.concatenate([gxa, gxb, rl16], axis=3)   # [nc, nblk, 128, BW]
    megas = megas.transpose(0, 2, 1, 3).reshape(NCORE, P, -1)
    stjs = stj.transpose(0, 2, 1, 3).reshape(NCORE, P, -1)
    iota = np.tile(np.arange(P, dtype=np.float32), (P, 1)).astype(ml_dtypes.bfloat16)
    reps = {
        "w1p": w1pack.astype(np.float32),
        "w2t": np.ascontiguousarray(W2),
        "vu2": np.ascontiguousarray(vu2p),
        "w3p": w3pack.astype(np.float32),
        "b1r": np.tile(b1, (P, 1)).astype(np.float32),
        "b2r": np.tile(b2, (P, 1)).astype(np.float32),
        "b3r": np.tile(b3, (P, 1)).astype(np.float32),
        "iot": iota,
    }

    in_maps = []
    for k in range(NCORE):
        xk = x[k * npc:(k + 1) * npc]
        xT = np.zeros((P, npcp), np.float32)
        xT[:, :npc] = xk.T
        m = {"xT": xT, "mega": np.ascontiguousarray(megas[k]),
             "stjt": np.ascontiguousarray(stjs[k])}
        m.update(reps)
        in_maps.append(m)
    return in_maps, SA, SB, npc, nblk


def _run(inputs, n, e, trace=False):
    from concourse.bass_utils import run_bass_kernel_spmd

    in_maps, SA, SB, npc, nblk = _host_inputs(inputs, n)
    nc = _build_program(n, SA, SB, npc, nblk)
    res = run_bass_kernel_spmd(
        nc, in_maps, core_ids=list(range(NCORE)), trace=trace)
    outs = [res.results[k]["out"] for k in range(NCORE)]
    full = np.concatenate(outs, axis=0).astype(np.float32)
    return full, res


def kernel(**inputs):
    full, _ = _run(inputs, N, E, trace=False)
    return full


# revision 9
# speedup vs baseline: 1.8449x; 1.1110x over previous
"""Trainium2 Bass kernel for a 3-layer GAT (nn_GATModel_32229434589362).

Strategy (dst-sharded, edge-major S-matrix aggregation, Ant dma_gather):
  - Nodes sharded by destination across 8 cores (6250/core); each core owns
    all edges with dst in its range (host-side bucketing of edge_index only).
  - Per layer a per-node gather table holds [per-head features | a_src]
    (L1/L2: 64xbf16 rows, 256B pair stride; L3: [h3 128 | asrc3 4] in
    192xbf16 rows, 768B pair stride), AllGather-replicated.
  - Gathers use InstDMAGatherAnt (int16 idx). The 50000-row range is covered
    by splitting each block's edges into src-even / src-odd groups gathered
    from parity-packed views of the same table (idx = src>>1 < 25000).
  - a_dst per edge is NOT gathered: a one-hot stj[j, (s p)] = (dst_rel==j)
    is precomputed on host (graph static, shared by all 3 layers) and the
    per-edge a_dst expands as per-subtile PE matmuls stj_s^T @ adst_blk
    into PSUM. adst tables are bf16 [nblk*128, 4] (block-padded).
  - p = exp(leaky(asrc+adst)) with NO max subtraction (scores are O(0.5);
    softmax is shift-invariant); normalization deferred: out = z/(s+1e-16).
  - Segment reduction per 128-node block: z,s accumulate over subtiles as PE
    matmuls with lhsT = S^T (bf16 indicator built by one is_equal) and
    rhs = [p*G | p].
"""
import sys

sys.path.insert(0, "/opt/trn_rl_repo")

import numpy as np

import concourse.bass as bass
import concourse.bacc as bacc
import concourse.tile as tile
import concourse.mybir as mybir
from concourse.masks import make_identity

f32 = mybir.dt.float32
bf16 = mybir.dt.bfloat16
i16 = mybir.dt.int16
AF = mybir.ActivationFunctionType
Alu = mybir.AluOpType

# problem constants
N, E, IN_C, H = 50000, 800000, 128, 4
NCORE, P = 8, 128
NEG = 0.2
EPS = 1e-16
R12 = 64        # T1/T2 row: [feat 32 | asrc 4 | zeros] bf16 (128B)
R3 = 192        # T3 row: [h3 128 | asrc3 4 | pad] bf16 (384B, pair stride 768B)
R3V = 132       # T3 gathered payload elems
PAD_REL = 200.0


# ---------------------------------------------------------------- host side

def _make_vu(W, att, heads, c):
    return np.stack(
        [W[:, h * c:(h + 1) * c] @ att[h] for h in range(heads)], axis=1
    ).astype(np.float32)


def _wrap16(vals):
    """int16 idx list -> [128, ceil(n/16)] wrapped layout: idx i at
    [i%16, i//16], replicated down all 8 groups of 16 partitions."""
    n = vals.shape[0]
    w = -(-n // 16)
    a = np.zeros((16, w), np.int16)
    a[(np.arange(n) % 16), (np.arange(n) // 16)] = vals.astype(np.int16)
    return np.tile(a, (8, 1))


def _preprocess(edge_index, n, ncore):
    """Bucket edges by (dst core, dst block), split each bucket by src parity.

    Slot (p, s) of a block = edge list position s*128+p. Group A (src even)
    occupies subtiles [0, SA), group B (odd) [SA, SA+SB). Gather idx is
    src>>1 (< 25000, fits int16); table rows are parity-packed in pairs.
      gxa [nblk, 128, SA*8] int16  (gather idx, wrapped layout, pad=0)
      gxb [nblk, 128, SB*8] int16
      rel [nblk, 128, ST] f32      (dst_rel, pad=PAD_REL)
      stj [nblk, 128, ST*128] bf16 (one-hot: stj[j, s*128+p] = rel[p,s]==j)
    """
    import ml_dtypes
    npc = n // ncore
    nblk = (npc + P - 1) // P
    src = np.asarray(edge_index[0], np.int64)
    dst = np.asarray(edge_index[1], np.int64)
    core = dst // npc
    dloc = dst % npc
    blk = dloc // P
    rel = (dloc % P).astype(np.float32)
    grp = (src & 1).astype(np.int64)

    nb = ncore * nblk
    key = (core * nblk + blk) * 2 + grp
    counts = np.bincount(key, minlength=nb * 2).reshape(nb, 2)
    SA = int(-(-counts[:, 0].max() // P))
    SB = int(-(-counts[:, 1].max() // P))
    ST = SA + SB

    # slot position within the block for each edge
    order = np.argsort(key, kind="stable")
    ksort = key[order]
    starts = np.zeros(nb * 2 + 1, np.int64)
    np.cumsum(counts.reshape(-1), out=starts[1:])
    slot_in_grp = np.arange(src.shape[0]) - starts[ksort]
    bucket = ksort // 2
    grp_s = ksort % 2
    pos = np.where(grp_s == 0, slot_in_grp, SA * P + slot_in_grp)
    flat = bucket * (ST * P) + pos

    gx = np.zeros(nb * ST * P, np.int64)
    rl = np.full(nb * ST * P, PAD_REL, np.float32)
    sv = src[order]
    gx[flat] = sv >> 1
    rl[flat] = rel[order]

    gx = gx.reshape(ncore, nblk, ST * P)
    rl = rl.reshape(ncore, nblk, ST, P)      # [c, b, s, p]

    gxa = np.zeros((ncore, nblk, 128, SA * 8), np.int16)
    gxb = np.zeros((ncore, nblk, 128, max(SB, 1) * 8), np.int16)
    for k in range(ncore):
        for b in range(nblk):
            gxa[k, b] = _wrap16(gx[k, b, :SA * P])
            if SB:
                gxb[k, b] = _wrap16(gx[k, b, SA * P:])

    # stj one-hot [c, b, j, (s p)]: edge at slot (p, s) with dst_rel j
    stj = np.zeros((ncore, nblk, P, ST * P), ml_dtypes.bfloat16)
    ci, bi = np.divmod(np.arange(nb)[bucket], nblk)
    pp = (pos % P).astype(np.int64)
    ss = (pos // P).astype(np.int64)
    jj = rel[order].astype(np.int64)
    stj[ci, bi, jj, ss * P + pp] = 1.0

    # rel as [c, b, p, s] (partition-major slots) for the device-side st
    rl = np.ascontiguousarray(rl.transpose(0, 1, 3, 2))
    return gxa, gxb, rl, stj, SA, SB, npc, nblk


# ---------------------------------------------------------------- device side

def _gather_calls(gp, out3, table_even, table_odd, idxa, idxb,
                  SA, SB, elem, elem_step):
    """2 calls per parity group (ceil/floor split, <=1024 descs each).
    Queues 1/2 (async posts), balanced so each queue gets c1+rest descs.
    out3: [128, ST, elem]."""
    qmap = (1, 2, 3, 0)
    pos = 0
    for base, cnt, tbl, idx in ((0, SA, table_even, idxa),
                                (SA, SB, table_odd, idxb)):
        c1 = (cnt + 1) // 2
        for c0, cl in ((0, c1), (c1, cnt - c1)):
            if cl <= 0:
                pos += 1
                continue
            assert cl * P <= 1024, "SWDGE ring limit"
            num = cl * P
            _dma_gather_raw(
                gp,
                out3[:, base + c0:base + c0 + cl, :],
                tbl,
                idx[:, c0 * 8:(c0 + cl) * 8],
                num, elem, elem_step,
                queue_num=qmap[pos],
            )
            pos += 1


def _dma_gather_raw(gp, out_ap, in_ap, idxs_ap, num_idxs, elem_size,
                    elem_step, queue_num=0):
    """dma_gather (DRAM, non-transpose) minus the elem_size%256 assert — the
    Q7 ucode is size-agnostic here; only the stride must be 256B-aligned."""
    from concourse import ap_utils
    assert idxs_ap.dtype == mybir.dt.int16
    assert in_ap.dtype == out_ap.dtype
    assert ap_utils.ap_is_contiguous(out_ap.ap[1:])
    assert ap_utils.ap_is_contiguous(idxs_ap.ap[1:])
    assert in_ap.ap[0][0] == elem_step and in_ap.ap[-1][1] == elem_size
    assert out_ap.ap[-1][1] == elem_size
    assert out_ap.ap[0][1] * out_ap.ap[1][1] == num_idxs
    stride_bytes_256 = (elem_step * mybir.dt.size(in_ap.dtype)) // 256
    assert stride_bytes_256 * 256 == elem_step * mybir.dt.size(in_ap.dtype)
    _in_ap = gp.lower_ap_dma(in_ap, for_custom_bir_dma=True)
    inst = gp.add_instruction(
        mybir.InstDMAGatherAnt(
            name=gp.bass.get_next_instruction_name(),
            ins=[*_in_ap, gp.lower_ap(idxs_ap),
                 gp.lower_val_access(gp.to_reg(num_idxs))],
            outs=[gp.lower_ap(out_ap)],
            transpose=False,
            num_idxs=num_idxs,
            elem_size=elem_size,
            stride_bytes_256=stride_bytes_256,
            gen_mode=0,
            single_packet=True,
            queue_num=queue_num,
            sbuf_tokens_per_rank=0,
            sbuf_free_dim_per_rank=0,
            sbuf_free_dim_pad_per_rank=0,
            sbuf_byte_offset=0,
        )
    )
    return inst


def _build_program(n, SA, SB, npc, nblk, use_collectives=True):
    nc = bacc.Bacc("TRN2", num_devices=NCORE, num_swdge_queues=4)
    ST = SA + SB
    npcp = nblk * P
    WA, WB = SA * 8, max(SB, 1) * 8

    BW = WA + WB + ST        # int16 cols per block in the mega index tile
    xT = nc.dram_tensor("xT", [P, npcp], f32, kind="ExternalInput")
    mega = nc.dram_tensor("mega", [P, nblk * BW], i16, kind="ExternalInput")
    stjt = nc.dram_tensor("stjt", [P, nblk * ST * P], bf16,
                          kind="ExternalInput")
    w1p = nc.dram_tensor("w1p", [P, 68], f32, kind="ExternalInput")
    w2t = nc.dram_tensor("w2t", [32, 128], f32, kind="ExternalInput")
    vu2 = nc.dram_tensor("vu2", [32, 8], f32, kind="ExternalInput")
    w3p = nc.dram_tensor("w3p", [P, 136], f32, kind="ExternalInput")
    b1r = nc.dram_tensor("b1r", [P, 32], f32, kind="ExternalInput")
    b2r = nc.dram_tensor("b2r", [P, 128], f32, kind="ExternalInput")
    b3r = nc.dram_tensor("b3r", [P, 32], f32, kind="ExternalInput")
    iot = nc.dram_tensor("iot", [P, P], bf16, kind="ExternalInput")
    out = nc.dram_tensor("out", [npc, 32], f32, kind="ExternalOutput")
    rg = [list(range(NCORE))]

    with tile.TileContext(nc) as tc:
        with tc.tile_pool(name="dramp", bufs=1, space="DRAM") as dramp, \
                tc.tile_pool(name="constp", bufs=1) as constp:
            t1loc = dramp.tile([npc, R12], bf16)
            t1full = dramp.tile([n, R12], bf16, addr_space="Shared")
            t2loc = dramp.tile([npc, R12], bf16)
            t2full = dramp.tile([n, R12], bf16, addr_space="Shared")
            t3loc = dramp.tile([npc, R3], bf16)
            t3full = dramp.tile([n, R3], bf16, addr_space="Shared")
            ad1 = dramp.tile([npcp, 4], bf16)
            ad2 = dramp.tile([npcp, 4], bf16)
            ad3 = dramp.tile([npcp, 4], bf16)

            def cload(name, shape, dt, src):
                t = constp.tile(shape, dt, name=name)
                nc.sync.dma_start(t, src)
                return t

            w1p_s = cload("w1p_s", [P, 68], f32, w1p[:, :])
            w2t_s = cload("w2t_s", [32, 128], f32, w2t[:, :])
            vu2_s = cload("vu2_s", [32, 8], f32, vu2[:, :])
            w3p_s = cload("w3p_s", [P, 136], f32, w3p[:, :])
            b1r_s = cload("b1r_s", [P, 32], f32, b1r[:, :])
            b2r_s = cload("b2r_s", [P, 128], f32, b2r[:, :])
            b3r_s = cload("b3r_s", [P, 32], f32, b3r[:, :])
            iot_s = cload("iot_s", [P, P], bf16, iot[:, :])
            ident = constp.tile([P, P], f32)
            make_identity(nc, ident)
            mega_s = constp.tile([P, nblk * BW], i16)
            nc.sync.dma_start(mega_s, mega[:, :])

            # parity-packed views: row pair (2g, 2g+1) -> packed row g; the
            # even/odd views address each half at stride 2*r elems
            def parity_views(t, r, v):
                w = t.rearrange("(a b) c -> a (b c)", b=2)
                return w[:, 0:v], w[:, r:r + v]

            t1a, t1b = parity_views(t1full, R12, R12)
            t2a, t2b = parity_views(t2full, R12, R12)
            t3a, t3b = parity_views(t3full, R3, R3V)

            # ---- stage A: T1 rows = [x@W1 | x@V1 | 0...], ad1 = x@U1 ----
            with tc.tile_pool(name="sa", bufs=3) as sa, \
                    tc.tile_pool(name="pa", bufs=2, space="PSUM") as pa:
                for b in range(nblk):
                    xb = sa.tile([P, P], f32, tag="xb")
                    nc.sync.dma_start(xb, xT[:, b * P:(b + 1) * P])
                    hp = pa.tile([P, 68], f32, tag="hp")
                    nc.tensor.matmul(hp, xb, w1p_s, start=True, stop=True)
                    hs = sa.tile([P, 64], bf16, tag="hs")
                    nc.vector.memset(hs[:, 36:64], 0.0)
                    nc.scalar.copy(hs[:, 0:36], hp[:, 0:36])
                    ha4 = sa.tile([P, 4], bf16, tag="ha4")
                    nc.scalar.copy(ha4, hp[:, 64:68])
                    rows = min(P, npc - b * P)
                    nc.sync.dma_start(
                        t1loc[b * P:b * P + rows, :], hs[0:rows, :])
                    nc.sync.dma_start(ad1[b * P:(b + 1) * P, :], ha4)

            def allgather(loc, full):
                if use_collectives:
                    nc.gpsimd.collective_compute(
                        "AllGather", Alu.bypass, replica_groups=rg,
                        ins=[loc[:, :].opt()], outs=[full[:, :].opt()])
                else:
                    nc.sync.dma_start(full[0:npc, :], loc[:, :])

            allgather(t1loc, t1full)

            def elu(sp, x_ap, cols, tag):
                mn = sp.tile([P, cols], f32, tag=tag + "mn")
                nc.vector.tensor_scalar_min(mn, x_ap, 0.0)
                ex = sp.tile([P, cols], f32, tag=tag + "ex")
                nc.scalar.activation(ex, mn, AF.Exp)
                mx = sp.tile([P, cols], f32, tag=tag + "mx")
                nc.vector.tensor_scalar_max(mx, x_ap, 0.0)
                sm = sp.tile([P, cols], f32, tag=tag + "sm")
                nc.vector.tensor_tensor(out=sm, in0=mx, in1=ex, op=Alu.add)
                res = sp.tile([P, cols], f32, tag=tag + "rs")
                nc.vector.tensor_scalar_add(res, sm, -1.0)
                return res

            def gat_layer(layer, tha, thb, adt, rpg, epilogue):
                """layer: 1/2/3. rpg: PG width (32 L1, 128 L2/L3)."""
                l3 = layer == 3
                Rt = R3V if l3 else R12
                estep = 2 * R3 if l3 else 2 * R12
                with tc.tile_pool(name=f"sp{layer}", bufs=4) as sp, \
                        tc.tile_pool(name=f"pp{layer}", bufs=2, space="PSUM") as pp:
                    for b in range(nblk):
                        base = b * BW
                        ixa = mega_s[:, base:base + WA]
                        ixb = mega_s[:, base + WA:base + WA + WB]
                        relf = mega_s[:, base + WA + WB:
                                      base + BW].bitcast(bf16)

                        G = sp.tile([P, ST * Rt], bf16, tag="G")
                        g3 = G.rearrange("p (s r) -> p s r", r=Rt)
                        _gather_calls(nc.gpsimd, g3, tha, thb,
                                      ixa, ixb, SA, SB, Rt, estep)

                        # stj one-hot [j, (s p)] + adst block tile
                        SJ = sp.tile([P, ST * P], bf16, tag="SJ")
                        nc.sync.dma_start(
                            SJ, stjt[:, b * ST * P:(b + 1) * ST * P])
                        sj3 = SJ.rearrange("p (s q) -> p s q", q=P)
                        adb = sp.tile([P, 4], bf16, tag="adb")
                        nc.sync.dma_start(adb, adt[b * P:(b + 1) * P, :])

                        # ADe[p, s, h] = sum_j stj[j, (s p)] * adst[j, h]
                        eep = pp.tile([P, ST * 4], f32, tag="eep", bufs=2)
                        for s in range(ST):
                            nc.tensor.matmul(
                                eep[:, 4 * s:4 * s + 4], sj3[:, s, :], adb,
                                start=True, stop=True)

                        # e = leaky(asrc + adst); p = exp(e)
                        ee = sp.tile([P, ST * 4], f32, tag="ee")
                        a0 = 128 if l3 else 32
                        nc.vector.tensor_tensor(
                            out=ee.rearrange("p (s h) -> p s h", h=4),
                            in0=g3[:, :, a0:a0 + 4],
                            in1=eep.rearrange("p (s h) -> p s h", h=4),
                            op=Alu.add)
                        es = sp.tile([P, ST * 4], f32, tag="es")
                        nc.vector.tensor_scalar_mul(es, ee, NEG)
                        el = sp.tile([P, ST * 4], f32, tag="el")
                        nc.vector.tensor_tensor(out=el, in0=ee, in1=es, op=Alu.max)
                        pt = sp.tile([P, ST * 4], bf16, tag="pt")
                        nc.scalar.activation(pt, el, AF.Exp)

                        # S^T[p, s, j] = (dst_rel == j), bf16
                        st = sp.tile([P, ST * P], bf16, tag="st")
                        nc.vector.tensor_tensor(
                            out=st.rearrange("p (s j) -> p s j", j=P),
                            in0=relf.unsqueeze(2).broadcast_to([P, ST, P]),
                            in1=iot_s.unsqueeze(1).broadcast_to([P, ST, P]),
                            op=Alu.is_equal)

                        # rhs = [p*G-slices | p] bf16 per subtile
                        rw = rpg + 4
                        pgp = sp.tile([P, ST * rw], bf16, tag="pgp")
                        pg4 = pgp.rearrange("p (s m) -> p s m", m=rw)
                        pt3 = pt.rearrange("p (s h) -> p s h", h=4)
                        cw = rpg // 4
                        if layer == 1:
                            in0 = g3[:, :, 0:32].rearrange(
                                "p s (h c) -> p s h c", c=8)
                        elif layer == 2:
                            in0 = g3[:, :, 0:32].unsqueeze(2).broadcast_to(
                                [P, ST, 4, 32])
                        else:
                            in0 = g3[:, :, 0:128].rearrange(
                                "p s (h c) -> p s h c", c=32)
                        nc.vector.tensor_tensor(
                            out=pg4[:, :, 0:rpg].rearrange(
                                "p s (h c) -> p s h c", c=cw),
                            in0=in0,
                            in1=pt3.unsqueeze(3).broadcast_to([P, ST, 4, cw]),
                            op=Alu.mult)
                        nc.vector.tensor_copy(out=pg4[:, :, rpg:rw], in_=pt3)

                        zb = pp.tile([P, rw], f32, tag="zb")
                        st3 = st.rearrange("p (s j) -> p s j", j=P)
                        for s in range(ST):
                            nc.tensor.matmul(
                                zb, st3[:, s, :], pg4[:, s, :],
                                start=(s == 0), stop=(s == ST - 1))

                        # znorm = z / (s + eps)
                        rr = sp.tile([P, 4], f32, tag="rr")
                        nc.vector.tensor_scalar_add(rr, zb[:, rpg:rw], EPS)
                        rr2 = sp.tile([P, 4], f32, tag="rr2")
                        nc.vector.reciprocal(rr2, rr)
                        zn = sp.tile([P, rpg], f32, tag="zn")
                        nc.vector.tensor_tensor(
                            out=zn.rearrange("p (h c) -> p h c", c=cw),
                            in0=zb[:, 0:rpg].rearrange("p (h c) -> p h c", c=cw),
                            in1=rr2.unsqueeze(2).broadcast_to([P, 4, cw]),
                            op=Alu.mult)

                        epilogue(b, sp, pp, zn)

            # ---- layer epilogues ----
            def epi1(b, sp, pp, zn):
                tb = sp.tile([P, 32], f32, tag="tb")
                nc.vector.tensor_tensor(out=tb, in0=zn, in1=b1r_s, op=Alu.add)
                t2 = elu(sp, tb, 32, "e1")
                t2T_p = pp.tile([32, P], f32, tag="t2T_p", bufs=1)
                nc.tensor.transpose(t2T_p, t2, ident)
                t2T = sp.tile([32, P], f32, tag="t2T")
                nc.scalar.copy(t2T, t2T_p)
                av_p = pp.tile([P, 8], f32, tag="av_p", bufs=1)
                nc.tensor.matmul(av_p, t2T, vu2_s, start=True, stop=True)
                stg = sp.tile([P, 64], bf16, tag="stg")
                nc.vector.memset(stg[:, 36:64], 0.0)
                nc.scalar.copy(stg[:, 0:32], t2)
                nc.scalar.copy(stg[:, 32:36], av_p[:, 0:4])
                avb = sp.tile([P, 4], bf16, tag="avb")
                nc.scalar.copy(avb, av_p[:, 4:8])
                rows = min(P, npc - b * P)
                nc.sync.dma_start(t2loc[b * P:b * P + rows, :], stg[0:rows, :])
                nc.sync.dma_start(ad2[b * P:(b + 1) * P, :], avb)

            def epi2(b, sp, pp, zn):
                o2 = pp.tile([P, 128], f32, tag="o2", bufs=1)
                for h in range(4):
                    zT_p = pp.tile([32, P], f32, tag="zT_p", bufs=1)
                    nc.tensor.transpose(zT_p, zn[:, 32 * h:32 * h + 32], ident)
                    zT = sp.tile([32, P], f32, tag="zT")
                    nc.scalar.copy(zT, zT_p)
                    nc.tensor.matmul(
                        o2[:, 32 * h:32 * h + 32], zT,
                        w2t_s[:, 32 * h:32 * h + 32], start=True, stop=True)
                ob = sp.tile([P, 128], f32, tag="ob")
                nc.vector.tensor_tensor(out=ob, in0=o2, in1=b2r_s, op=Alu.add)
                t3 = elu(sp, ob, 128, "e2")
                t3T_p = pp.tile([P, P], f32, tag="t3T_p", bufs=1)
                nc.tensor.transpose(t3T_p, t3, ident)
                t3T = sp.tile([P, P], f32, tag="t3T")
                nc.scalar.copy(t3T, t3T_p)
                hv_p = pp.tile([P, 136], f32, tag="hv_p", bufs=1)
                nc.tensor.matmul(hv_p, t3T, w3p_s, start=True, stop=True)
                hvb = sp.tile([P, R3V], bf16, tag="hvb")
                nc.scalar.copy(hvb, hv_p[:, 0:R3V])
                hva = sp.tile([P, 4], bf16, tag="hva")
                nc.scalar.copy(hva, hv_p[:, 132:136])
                rows = min(P, npc - b * P)
                nc.sync.dma_start(
                    t3loc[b * P:b * P + rows, 0:R3V], hvb[0:rows, :])
                nc.sync.dma_start(ad3[b * P:(b + 1) * P, :], hva)

            def epi3(b, sp, pp, zn):
                a1 = sp.tile([P, 32], f32, tag="a1")
                nc.vector.tensor_tensor(
                    out=a1, in0=zn[:, 0:32], in1=zn[:, 32:64], op=Alu.add)
                a2 = sp.tile([P, 32], f32, tag="a2")
                nc.vector.tensor_tensor(
                    out=a2, in0=a1, in1=zn[:, 64:96], op=Alu.add)
                a3 = sp.tile([P, 32], f32, tag="a3")
                nc.vector.tensor_tensor(
                    out=a3, in0=a2, in1=zn[:, 96:128], op=Alu.add)
                a4 = sp.tile([P, 32], f32, tag="a4")
                nc.vector.tensor_scalar_mul(a4, a3, 0.25)
                a5 = sp.tile([P, 32], f32, tag="a5")
                nc.vector.tensor_tensor(out=a5, in0=a4, in1=b3r_s, op=Alu.add)
                rows = min(P, npc - b * P)
                nc.sync.dma_start(out[b * P:b * P + rows, :], a5[0:rows, :])

            gat_layer(1, t1a, t1b, ad1, 32, epi1)
            allgather(t2loc, t2full)
            gat_layer(2, t2a, t2b, ad2, 128, epi2)
            allgather(t3loc, t3full)
            gat_layer(3, t3a, t3b, ad3, 128, epi3)

    nc.compile()
    return nc


# ---------------------------------------------------------------- entry point

def _host_inputs(inputs, n):
    x = np.asarray(inputs["x"], np.float32)
    edge_index = np.asarray(inputs["edge_index"])
    W1 = np.asarray(inputs["W1"], np.float32)
    W2 = np.asarray(inputs["W2"], np.float32)
    W3 = np.asarray(inputs["W3"], np.float32)
    as1, ad1 = np.asarray(inputs["as1"]), np.asarray(inputs["ad1"])
    as2, ad2 = np.asarray(inputs["as2"]), np.asarray(inputs["ad2"])
    as3, ad3 = np.asarray(inputs["as3"]), np.asarray(inputs["ad3"])
    b1, b2, b3 = [np.asarray(inputs[k], np.float32) for k in ("b1", "b2", "b3")]

    gxa, gxb, rl, stj, SA, SB, npc, nblk = _preprocess(edge_index, n, NCORE)
    npcp = nblk * P

    # w1p: [W1(32) | V1(4) | zeros(28) | U1(4)] -> psum cols 0:64 = table row
    w1pack = np.concatenate(
        [W1, _make_vu(W1, as1, H, 8), np.zeros((IN_C, 28), np.float32),
         _make_vu(W1, ad1, H, 8)], axis=1)
    vu2p = np.concatenate(
        [_make_vu(W2, as2, H, 32), _make_vu(W2, ad2, H, 32)], axis=1)
    # w3p: [W3 (128) | V3=as3 proj (4) | U3=ad3 proj (4)]
    w3pack = np.concatenate(
        [W3, _make_vu(W3, as3, H, 32), _make_vu(W3, ad3, H, 32)], axis=1)

    import ml_dtypes
    ST = SA + SB
    WA, WB = SA * 8, max(SB, 1) * 8
    rl16 = rl.astype(ml_dtypes.bfloat16).view(np.int16)
    megas = np.concatenate([gxa, gxb, rl16], axis=3)   # [nc, nblk, 128, BW]
    megas = megas.transpose(0, 2, 1, 3).reshape(NCORE, P, -1)
    stjs = stj.transpose(0, 2, 1, 3).reshape(NCORE, P, -1)
    iota = np.tile(np.arange(P, dtype=np.float32), (P, 1)).astype(ml_dtypes.bfloat16)
    reps = {
        "w1p": w1pack.astype(np.float32),
        "w2t": np.ascontiguousarray(W2),
        "vu2": np.ascontiguousarray(vu2p),
        "w3p": w3pack.astype(np.float32),
        "b1r": np.tile(b1, (P, 1)).astype(np.float32),
        "b2r": np.tile(b2, (P, 1)).astype(np.float32),
        "b3r": np.tile(b3, (P, 1)).astype(np.float32),
        "iot": iota,
    }

    in_maps = []
    for k in range(NCORE):
        xk = x[k * npc:(k + 1) * npc]
        xT = np.zeros((P, npcp), np.float32)
        xT[:, :npc] = xk.T
        m = {"xT": xT, "mega": np.ascontiguousarray(megas[k]),
             "stjt": np.ascontiguousarray(stjs[k])}
        m.update(reps)
        in_maps.append(m)
    return in_maps, SA, SB, npc, nblk


def _run(inputs, n, e, trace=False):
    from concourse.bass_utils import run_bass_kernel_spmd

    in_maps, SA, SB, npc, nblk = _host_inputs(inputs, n)
    nc = _build_program(n, SA, SB, npc, nblk)
    res = run_bass_kernel_spmd(
        nc, in_maps, core_ids=list(range(NCORE)), trace=trace)
    outs = [res.results[k]["out"] for k in range(NCORE)]
    full = np.concatenate(outs, axis=0).astype(np.float32)
    return full, res


def kernel(**inputs):
    full, _ = _run(inputs, N, E, trace=False)
    return full
